# revision 1
# baseline (speedup 1.0000x reference)
"""Swin-style windowed attention (B_=2048 windows, N=49 tokens, C=512, 16 heads)
on 8 Trainium2 NeuronCores, data-parallel over windows (256 windows/core).

Layout strategy (per core):
  - host pre-transposes x -> xT [C, 12560] bf16 (feature-major, 16-token
    zero pad), pre-transposes weights, folds the 1/sqrt(hd) scale into the q
    columns, and precomputes exp(relative-position-bias) tiles.
  - q,k computed feature-major (head h lands at partition 32*(h%4) of tile
    h//4), v computed token-major with the two windows of a pair at partition
    slots {0, 64} so window rows are 32-aligned for tile_position addressing.
  - scores^T via K=32 row-packed matmuls; matmuls with different tile_position
    rows must target different PSUM banks (HW constraint), so scores use one
    PSUM tile per row group (4 heads each) and AV uses one PSUM tile per
    window slot.
  - rel-pos bias applied as multiplicative exp(bias) on GPSIMD after the ACT
    exp; AV + softmax denominator fused in one matmul per (window, head) via a
    v tile with interleaved ones columns (N=33); normalization = DVE
    reciprocal + free-broadcast multiply; attention output transposed back to
    feature-major on the PE for the final projection.
"""

import os
import sys

import numpy as np
import ml_dtypes

if "/opt/trn_rl_repo" not in sys.path:
    sys.path.insert(0, "/opt/trn_rl_repo")

P = 49          # tokens per window
NH = 16         # heads
HD = 32         # head dim
C = 512         # model dim
NCORES = 8
B_TOTAL = 2048
B_CORE = B_TOTAL // NCORES        # 256 windows per core
TOK = B_CORE * P                  # 12544 tokens per core
OCT = 32                          # octets (8 windows) per core
OCT_TOK = 8 * P                   # 392 tokens per octet
OCT_W = OCT_TOK + 16              # octet tile width incl 16-token overlap
TOK_PAD = TOK + 16
BF16 = ml_dtypes.bfloat16


def _build(nc, has_bqk, has_bv, has_bp, n_oct=OCT):
    import concourse.bass as bass
    import concourse.mybir as mybir
    from concourse.tile import TileContext
    from concourse.masks import make_identity

    F32 = mybir.dt.float32
    BF = mybir.dt.bfloat16
    Exp = mybir.ActivationFunctionType.Exp

    xT = nc.dram_tensor("xT", [C, TOK_PAD], BF, kind="ExternalInput")
    wqk = nc.dram_tensor("wqk", [C, 2 * C], BF, kind="ExternalInput")
    wv = nc.dram_tensor("wv", [C, C], BF, kind="ExternalInput")
    wp = nc.dram_tensor("wp", [C, C], BF, kind="ExternalInput")
    eb = nc.dram_tensor("eb", [128, 4, 4, P], F32, kind="ExternalInput")
    bqk = bv = bp = None
    if has_bqk:
        bqk = nc.dram_tensor("bqk", [1, 2 * C], BF, kind="ExternalInput")
    if has_bv:
        bv = nc.dram_tensor("bv", [1, C], BF, kind="ExternalInput")
    if has_bp:
        bp = nc.dram_tensor("bp", [1, C], BF, kind="ExternalInput")
    out = nc.dram_tensor("out", [TOK, C], BF, kind="ExternalOutput")

    def bcast_last(ap, n):
        return bass.AP(ap.tensor, ap.offset, [*ap.ap, [0, n]])

    with TileContext(nc) as tc:
        with (
            tc.tile_pool(name="singles", bufs=1) as singles,
            tc.tile_pool(name="xt", bufs=2) as xt_pool,
            tc.tile_pool(name="qk", bufs=2) as qk_pool,
            tc.tile_pool(name="vsb", bufs=8) as v_pool,
            tc.tile_pool(name="se", bufs=3) as se_pool,
            tc.tile_pool(name="zr", bufs=4) as zr_pool,
            tc.tile_pool(name="attn", bufs=6) as attn_pool,
            tc.tile_pool(name="att", bufs=4) as atT_pool,
            tc.tile_pool(name="osb", bufs=4) as out_pool,
            tc.tile_pool(name="ps_big", bufs=2, space="PSUM") as ps_big,
            tc.tile_pool(name="ps_st", bufs=1, space="PSUM") as ps_st,
            tc.tile_pool(name="ps_av", bufs=1, space="PSUM") as ps_av,
        ):
            # --- constants / weights ---
            wqk_sb = []
            wv_sb = []
            wp_sb = []
            for ci in range(4):
                wqk_t = singles.tile([128, 2 * C], BF, name=f"wqk{ci}")
                nc.sync.dma_start(out=wqk_t, in_=wqk[128 * ci:128 * (ci + 1), :])
                wqk_sb.append(wqk_t)
                wv_t = singles.tile([128, C], BF, name=f"wv{ci}")
                nc.sync.dma_start(out=wv_t, in_=wv[128 * ci:128 * (ci + 1), :])
                wv_sb.append(wv_t)
                wp_t = singles.tile([128, C], BF, name=f"wp{ci}")
                nc.sync.dma_start(out=wp_t, in_=wp[128 * ci:128 * (ci + 1), :])
                wp_sb.append(wp_t)
            eb_sb = singles.tile([128, 4, 4, P], F32, name="ebsb")
            nc.sync.dma_start(out=eb_sb, in_=eb[:, :, :, :])
            ident = singles.tile([128, 128], BF, name="ident")
            make_identity(nc, ident)
            bqk_sb = bv_sb = bp_sb = ones_row = None
            if has_bqk or has_bv or has_bp:
                ones_row = singles.tile([1, OCT_W], BF, name="onesrow")
                nc.vector.memset(ones_row, 1.0)
            if has_bqk:
                bqk_sb = singles.tile([1, 2 * C], BF, name="bqksb")
                nc.sync.dma_start(out=bqk_sb, in_=bqk[:, :])
            if has_bv:
                bv_sb = singles.tile([1, C], BF, name="bvsb")
                nc.sync.dma_start(out=bv_sb, in_=bv[:, :])
            if has_bp:
                bp_sb = singles.tile([1, C], BF, name="bpsb")
                nc.sync.dma_start(out=bp_sb, in_=bp[:, :])

            # --- main loop ---
            for o in range(n_oct):
                t0 = o * OCT_TOK
                xts = []
                xt2s = []
                for ci in range(4):
                    xt_t = xt_pool.tile([128, OCT_W], BF, name=f"xt{o}_{ci}",
                                        tag=f"xt{ci}")
                    nc.sync.dma_start(
                        out=xt_t,
                        in_=xT[128 * ci:128 * (ci + 1), t0:t0 + OCT_W])
                    xts.append(xt_t)
                    # slot-expanded copy for the v projection: per pair the
                    # two windows at 64-aligned positions ({0..63, 49..112})
                    xt2_t = xt_pool.tile([128, 4, 128], BF, name=f"xt2{o}_{ci}",
                                         tag=f"xt2{ci}")
                    xr = xT[128 * ci:128 * (ci + 1), :]
                    src_ap = bass.AP(xr.tensor, xr.offset + t0,
                                     [xr.ap[0], [98, 4], [P, 2], [1, 64]])
                    nc.sync.dma_start(out=xt2_t, in_=src_ap)
                    xt2s.append(xt2_t)

                # q,k feature-major: qks[0:4]=q tiles, qks[4:8]=k tiles
                qks = []
                for ft in range(8):
                    ps = ps_big.tile([128, OCT_W], F32, name=f"qkp{o}_{ft}",
                                     tag="big")
                    for ci in range(4):
                        nc.tensor.matmul(ps,
                                         wqk_sb[ci][:, 128 * ft:128 * (ft + 1)],
                                         xts[ci], start=(ci == 0),
                                         stop=(ci == 3 and not has_bqk))
                    if has_bqk:
                        nc.tensor.matmul(ps, bqk_sb[:, 128 * ft:128 * (ft + 1)],
                                         ones_row, start=False, stop=True)
                    sb = qk_pool.tile([128, OCT_W], BF, name=f"qk{o}_{ft}",
                                      tag=f"qk{ft}")
                    if ft < 4:
                        nc.scalar.copy(sb, ps)
                    else:
                        nc.vector.tensor_copy(sb, ps)
                    qks.append(sb)

                # phases 2+3 merged per pair: v chain is short
                # (matmuls -> one copy) so pairs pipeline through
                # the big pool without blocking attention
                v_sbs = []
                attn_sbs = []
                for p in range(4):
                    pt0 = 98 * p
                    vps = ps_big.tile([128, C], F32, name=f"vp{o}_{p}", tag="big")
                    for ci in range(4):
                        # both windows at slot-aligned output partitions 0/64
                        # in a single M=128 matmul (slot-expanded x copy)
                        nc.tensor.matmul(
                            vps, xt2s[ci][:, p, :],
                            wv_sb[ci], start=(ci == 0),
                            stop=(ci == 3 and not has_bv))
                    if has_bv:
                        nc.tensor.matmul(vps, ones_row[:, 0:128], bv_sb,
                                         start=False, stop=True)
                    v_sb = v_pool.tile([128, NH, 33], BF, name=f"v{o}_{p}", tag="v")
                    nc.scalar.copy(
                        v_sb[:, :, 0:32],
                        vps.rearrange("q (h d) -> q h d", h=NH))
                    nc.gpsimd.memset(v_sb[:, :, 32], 1.0)
                    v_sbs.append(v_sb)


                    pt0 = 98 * p
                    v_sb = v_sbs[p]
                    # scores^T: one PSUM tile per row group j (heads j, j+4,
                    # j+8, j+12); both window slots share the tile (same
                    # tile_position row, cols 0/64)
                    ses = []
                    for j in range(4):
                        stp = ps_st.tile([128, 4, P], F32,
                                         name=f"st{o}_{p}_{j}", tag=f"st{j}")
                        r = 32 * j
                        for i in range(4):
                            h = 4 * i + j
                            qq = qks[h // 4]
                            kk = qks[4 + h // 4]
                            for wi in range(2):
                                s = 64 * wi
                                wtok = pt0 + P * wi
                                nc.tensor.matmul(
                                    stp[s:s + 64, i, :],
                                    kk[r:r + 32, wtok:wtok + 64],
                                    qq[r:r + 32, wtok:wtok + P],
                                    start=True, stop=True,
                                    tile_position=(r, s))
                        se = se_pool.tile([128, 4, P], BF,
                                          name=f"se{o}_{p}_{j}", tag=f"se{j}")
                        nc.scalar.activation(out=se, in_=stp, func=Exp)
                        nc.gpsimd.tensor_mul(se, se, eb_sb[:, j, :, :])
                        ses.append(se)

                    attn_sb = attn_pool.tile([128, NH, HD], BF,
                                             name=f"attn{o}_{p}", tag="attn")
                    nc.gpsimd.memset(attn_sb, 0.0)   # define gap rows
                    # head h = 4a + c; AV pass G covers c in {2G, 2G+1} so it
                    # only needs se tiles j=2G, 2G+1 (starts before j=2G+2 is
                    # ready). attn_sb viewed [q, a, c, d] for scattered writes.
                    attn_v = attn_sb.rearrange("q (a c) d -> q a c d", a=4)
                    for G in range(2):
                        avs = []
                        for wi in range(2):
                            s = 64 * wi
                            av = ps_av.tile([128, 4, 2, 33], F32,
                                            name=f"av{o}_{p}_{G}_{wi}",
                                            tag=f"av{wi}")
                            for a in range(4):
                                for cc in range(2):
                                    c = 2 * G + cc
                                    h = 4 * a + c
                                    nc.tensor.matmul(
                                        av[s:s + P, a, cc, :],
                                        ses[c][s:s + P, a, :],
                                        v_sb[s:s + P, h, :],
                                        start=True, stop=True,
                                        tile_position=(s, s))
                            avs.append(av)
                        zr = zr_pool.tile([128, 4, 2], F32,
                                          name=f"zr{o}_{p}_{G}", tag="zr")
                        for wi in range(2):
                            s = 64 * wi
                            av = avs[wi]
                            nc.vector.reciprocal(zr[s:s + P, :, :],
                                                 av[s:s + P, :, :, 32])
                            nc.vector.tensor_mul(
                                attn_v[s:s + P, :, 2 * G:2 * G + 2, :],
                                av[s:s + P, :, :, 0:32],
                                bcast_last(zr[s:s + P, :, :], HD))
                    attn_sbs.append(attn_sb)

                # phase 4: transpose + projection per pair
                for p in range(4):
                    gt0 = t0 + 98 * p
                    atp = ps_big.tile([128, 4, 128], BF, name=f"atp{o}_{p}",
                                      tag="big")
                    attn_flat = attn_sbs[p].rearrange("q h d -> q (h d)")
                    for ci in range(4):
                        nc.tensor.transpose(atp[:, ci, :],
                                            attn_flat[:, 128 * ci:128 * (ci + 1)],
                                            ident)
                    atT_sb = atT_pool.tile([128, 4, 128], BF, name=f"atT{o}_{p}",
                                           tag="atT")
                    nc.vector.tensor_copy(atT_sb, atp)

                    pso = ps_big.tile([128, C], F32, name=f"po{o}_{p}", tag="big")
                    for ci in range(4):
                        nc.tensor.matmul(pso, atT_sb[:, ci, :], wp_sb[ci],
                                         start=(ci == 0),
                                         stop=(ci == 3 and not has_bp))
                    if has_bp:
                        nc.tensor.matmul(pso, ones_row[:, 0:128], bp_sb,
                                         start=False, stop=True)
                    osb = out_pool.tile([128, C], BF, name=f"o{o}_{p}", tag="osb")
                    nc.scalar.copy(osb, pso)
                    nc.sync.dma_start(out=out[gt0:gt0 + P, :], in_=osb[0:P, :])
                    nc.sync.dma_start(out=out[gt0 + P:gt0 + 2 * P, :],
                                      in_=osb[64:64 + P, :])
    return nc


def _host_prep(x, qkv_w, qkv_b, proj_w, proj_b, rpb_table, rel_index):
    scale = HD ** -0.5
    # weights: qkv feature order is (3, NH, HD) -> q=0:512, k=512:1024, v=1024:1536
    wq = qkv_w[0:C, :] * scale          # fold scale into q
    wk = qkv_w[C:2 * C, :]
    wv = qkv_w[2 * C:3 * C, :]
    wqk = np.concatenate([wq.T, wk.T], axis=1).astype(BF16)     # [C, 2C]
    wv_t = np.ascontiguousarray(wv.T).astype(BF16)              # [C, C]
    wp_t = np.ascontiguousarray(proj_w.T).astype(BF16)          # [C, C]

    bias = rpb_table[rel_index]                  # [n, m, NH], attn[h,n,m] += bias[n,m,h]
    biasT = np.transpose(bias, (2, 1, 0))        # [h, m, n]
    ebias = np.exp(biasT.astype(np.float64)).astype(np.float32)
    ebp = np.ones((128, NH, P), np.float32)
    ebp[0:P] = np.transpose(ebias, (1, 0, 2))    # rows 0:49  (window slot 0)
    ebp[64:64 + P] = ebp[0:P]                    # rows 64:113 (window slot 1)
    # regroup heads: [128, j, i, P] with head 4*i + j at [:, j, i, :]
    eb = np.ascontiguousarray(
        ebp.reshape(128, 4, 4, P).transpose(0, 2, 1, 3))

    bq = qkv_b[0:C] * scale
    bk = qkv_b[C:2 * C]
    bv_ = qkv_b[2 * C:3 * C]
    bqk = np.concatenate([bq, bk])[None, :].astype(BF16)
    bv = bv_[None, :].astype(BF16)
    bp = proj_b[None, :].astype(BF16)

    has_bqk = bool(np.any(bqk != 0))
    has_bv = bool(np.any(bv != 0))
    has_bp = bool(np.any(bp != 0))

    in_maps = []
    for c in range(NCORES):
        xc = x[c * B_CORE:(c + 1) * B_CORE].reshape(TOK, C)
        xTc = np.zeros((C, TOK_PAD), BF16)
        xTc[:, :TOK] = xc.T.astype(BF16)
        m = {"xT": xTc, "wqk": wqk, "wv": wv_t, "wp": wp_t, "eb": eb}
        if has_bqk:
            m["bqk"] = bqk
        if has_bv:
            m["bv"] = bv
        if has_bp:
            m["bp"] = bp
        in_maps.append(m)
    return in_maps, has_bqk, has_bv, has_bp


def kernel(x, qkv_w, qkv_b, proj_w, proj_b, rpb_table, rel_index):
    from concourse import bacc
    from concourse.bass_utils import run_bass_kernel_spmd

    in_maps, has_bqk, has_bv, has_bp = _host_prep(
        np.asarray(x, np.float32), np.asarray(qkv_w, np.float32),
        np.asarray(qkv_b, np.float32), np.asarray(proj_w, np.float32),
        np.asarray(proj_b, np.float32), np.asarray(rpb_table, np.float32),
        np.asarray(rel_index))

    nc = bacc.Bacc()
    _build(nc, has_bqk, has_bv, has_bp)
    nc.finalize()

    trace = os.environ.get("BASS_KERNEL_TRACE", "") == "1"
    res = run_bass_kernel_spmd(nc, in_maps, core_ids=list(range(NCORES)),
                               trace=trace)
    if trace and res.exec_time_ns is not None:
        print(f"HW exec time: {res.exec_time_ns} ns", flush=True)

    outs = [r["out"].astype(np.float32).reshape(B_CORE, P, C)
            for r in res.results]
    return np.concatenate(outs, axis=0)



# revision 2
# speedup vs baseline: 1.1496x; 1.1496x over previous
"""Swin windowed attention (B_=2048 windows, N=49, C=512, 16 heads) on 8
Trainium2 cores, data-parallel over windows (256 windows/core).

v2 layout strategy (per core), derived from HW analysis of the v1 baseline
(PE sequencer saturated at ~28ns/instruction with 672 PE insts/octet, plus
LDWEIGHTS serialization on same-row-group score/AV matmuls):

  - QKV projection for q,k runs in fp8e4 with DoubleRow perf mode (K=256 per
    pass, 2 passes instead of 4): halves both instruction count and cycles.
    Quantization scales (q x256, k x64) are folded out in the exp's scale.
  - scores^T computed 2-windows-per-matmul: stationary k is a [32, 128]
    tile whose 128 columns are both windows of a pair (64-slot aligned,
    materialized contiguously in "kslot" tiles so FWL stays enabled);
    moving q covers both windows' tokens (98 cols). 64 score matmuls/octet
    instead of 128, at 4 rotating row groups so LDWEIGHTS overlaps.
  - exp'd scores land in zero-padded block-diagonal "sespad" tiles
    [128, 4i, 128]: window 0 at [0:49, i, 0:49], window 1 at
    [64:113, i, 64:113], zeros elsewhere (memset once per pool buffer;
    exp/eb only ever rewrite the two blocks).  One AV matmul per (pair,
    head) then contracts over all 128 partitions with the [128, 128]
    contiguous sespad stationary: 64 AV matmuls/octet instead of 128.
  - v projection keeps the slot-expanded xt2 DMA (windows at 64-slots) so
    its stationary is contiguous.
  - attention output is normalized (DVE reciprocal x free-broadcast mul),
    PE-transposed to feature-major, compacted to dense tokens by the
    projection matmul's moving AP, and the final projection is emitted
    feature-major: out^T [C, TOK] in DRAM, un-transposed on the host.
"""

import os
import sys

import numpy as np
import ml_dtypes

if "/opt/trn_rl_repo" not in sys.path:
    sys.path.insert(0, "/opt/trn_rl_repo")

P = 49          # tokens per window
NH = 16         # heads
HD = 32         # head dim
C = 512         # model dim
NCORES = 8
B_TOTAL = 2048
B_CORE = B_TOTAL // NCORES        # 256 windows per core
TOK = B_CORE * P                  # 12544 tokens per core
OCT = 32                          # octets (8 windows) per core
OCT_TOK = 8 * P                   # 392 tokens per octet
OCT_W = OCT_TOK + 16              # octet tile width incl 16-token overlap
OCT_W8 = OCT_TOK + 24             # fp8 tile width (dim-1 step must be %16)
TOK_PAD = TOK + 16
TOK_PAD8 = TOK + 32
BF16 = ml_dtypes.bfloat16
FP8 = ml_dtypes.float8_e4m3fn

USE_FP8 = True
SQ = 256.0     # fp8 scale folded into q weights (power of 2)
SK = 64.0      # fp8 scale folded into k weights
ESC = 1.0 / (SQ * SK) if USE_FP8 else 1.0


def _build(nc, has_bqk, has_bv, has_bp, n_oct=OCT):
    import concourse.bass as bass
    import concourse.mybir as mybir
    from concourse.tile import TileContext
    from concourse.masks import make_identity

    F32 = mybir.dt.float32
    BF = mybir.dt.bfloat16
    F8 = mybir.dt.float8e4
    Exp = mybir.ActivationFunctionType.Exp
    DR = mybir.MatmulPerfMode.DoubleRow

    if USE_FP8:
        x8 = nc.dram_tensor("x8", [128, 2, 2, TOK_PAD8], F8, kind="ExternalInput")
        w8 = nc.dram_tensor("w8", [2, 128, 2, 2 * C], F8, kind="ExternalInput")
    else:
        wqk = nc.dram_tensor("wqk", [C, 2 * C], BF, kind="ExternalInput")
    xT = nc.dram_tensor("xT", [C, TOK_PAD], BF, kind="ExternalInput")
    wv = nc.dram_tensor("wv", [C, C], BF, kind="ExternalInput")
    wp = nc.dram_tensor("wp", [C, C], BF, kind="ExternalInput")
    eb = nc.dram_tensor("eb", [128, 4, 4, P], BF, kind="ExternalInput")
    bqk = bv = bp = None
    if has_bqk:
        bqk = nc.dram_tensor("bqk", [1, 2 * C], BF, kind="ExternalInput")
    if has_bv:
        bv = nc.dram_tensor("bv", [1, C], BF, kind="ExternalInput")
    if has_bp:
        bp = nc.dram_tensor("bp", [1, C], BF, kind="ExternalInput")
    out = nc.dram_tensor("out", [C, TOK], BF, kind="ExternalOutput")

    def bcast_last(ap, n):
        return bass.AP(ap.tensor, ap.offset, [*ap.ap, [0, n]])

    with TileContext(nc) as tc:
        with (
            tc.tile_pool(name="singles", bufs=1) as singles,
            tc.tile_pool(name="xin", bufs=3) as x_pool,
            tc.tile_pool(name="qk", bufs=2) as qk_pool,
            tc.tile_pool(name="vsb", bufs=6) as v_pool,
            tc.tile_pool(name="se", bufs=3) as se_pool,
            tc.tile_pool(name="zr", bufs=8) as zr_pool,
            tc.tile_pool(name="attn", bufs=6) as attn_pool,
            tc.tile_pool(name="att", bufs=3) as atT_pool,
            tc.tile_pool(name="osb", bufs=4) as out_pool,
            tc.tile_pool(name="ps_head", bufs=2, space="PSUM") as ps_head,
            tc.tile_pool(name="ps_st", bufs=1, space="PSUM") as ps_st,
            tc.tile_pool(name="ps_avt", bufs=1, space="PSUM") as ps_avt,
            tc.tile_pool(name="ps_po", bufs=1, space="PSUM") as ps_po,
        ):
            # --- constants / weights ---
            if USE_FP8:
                w8_sb = []
                for ci in range(2):
                    w8_t = singles.tile([128, 2, 2 * C], F8, name=f"w8{ci}")
                    nc.sync.dma_start(out=w8_t, in_=w8[ci, :, :, :])
                    w8_sb.append(w8_t)
            else:
                wqk_sb = []
                for ci in range(4):
                    wqk_t = singles.tile([128, 2 * C], BF, name=f"wqk{ci}")
                    nc.sync.dma_start(out=wqk_t, in_=wqk[128 * ci:128 * (ci + 1), :])
                    wqk_sb.append(wqk_t)
            wv_sb = []
            wp_sb = []
            for ci in range(4):
                wv_t = singles.tile([128, C], BF, name=f"wv{ci}")
                nc.sync.dma_start(out=wv_t, in_=wv[128 * ci:128 * (ci + 1), :])
                wv_sb.append(wv_t)
                wp_t = singles.tile([128, C], BF, name=f"wp{ci}")
                nc.sync.dma_start(out=wp_t, in_=wp[128 * ci:128 * (ci + 1), :])
                wp_sb.append(wp_t)
            eb_sb = singles.tile([128, 4, 4, P], BF, name="ebsb")
            nc.sync.dma_start(out=eb_sb, in_=eb[:, :, :, :])
            ident = singles.tile([128, 128], BF, name="ident")
            make_identity(nc, ident)
            bqk_sb = bv_sb = bpT_sb = ones_row = None
            if has_bqk or has_bv or has_bp:
                ones_row = singles.tile([1, OCT_W], BF, name="onesrow")
                nc.vector.memset(ones_row, 1.0)
            if has_bqk:
                bqk_sb = singles.tile([1, 2 * C], BF, name="bqksb")
                nc.sync.dma_start(out=bqk_sb, in_=bqk[:, :])
            if has_bv:
                bv_sb = singles.tile([1, C], BF, name="bvsb")
                nc.sync.dma_start(out=bv_sb, in_=bv[:, :])
            if has_bp:
                bpT_sb = singles.tile([1, C], BF, name="bpsb")
                nc.sync.dma_start(out=bpT_sb, in_=bp[:, :])

            # pre-zero the block-diagonal ses tiles (exp/eb rewrite only the
            # two diagonal blocks; the zero padding is what makes the fused
            # 2-window AV stationary sound) and pre-set the v denominator
            # ones column (the v copy writes only [:, :, 0:32]).
            for b in range(3):
                sez = se_pool.tile([128, 4, 4, 128], BF, name=f"sez{b}",
                                   tag="se")
                nc.gpsimd.memset(sez, 0.0)
            for b in range(6):
                vz = v_pool.tile([128, NH, 33], BF, name=f"vz{b}", tag="v")
                nc.gpsimd.memset(vz, 1.0)

            # --- main loop ---
            for o in range(n_oct):
                t0 = o * OCT_TOK
                # input DMAs (SP ring only carries inputs; outputs go via
                # the Activation ring so next-octet loads are never queued
                # behind this octet's stores)
                if USE_FP8:
                    x8t = x_pool.tile([128, 2, 2, OCT_W8], F8,
                                      name=f"x8{o}", tag="x8")
                    x8a = x8[:, :, :, :]
                    src8 = bass.AP(
                        x8a.tensor, x8a.offset + t0,
                        [[4 * TOK_PAD8, 128], [TOK_PAD8, 4], [1, OCT_W8]])
                    nc.sync.dma_start(out=x8t, in_=src8)
                xtile = x_pool.tile([128, 4, OCT_W], BF, name=f"xt{o}",
                                    tag="xt")
                xTa = xT[:, :]
                srcx = bass.AP(
                    xTa.tensor, xTa.offset + t0,
                    [[TOK_PAD, 128], [128 * TOK_PAD, 4], [1, OCT_W]])
                nc.sync.dma_start(out=xtile, in_=srcx)
                xts = [xtile[:, ci, :] for ci in range(4)]
                # slot-expanded copies for the V stationary (weights APs may
                # only have one free dim); SBUF->SBUF so Pool can do them
                xvs = []
                for ci in range(4):
                    xv = x_pool.tile([128, 4, 128], BF, name=f"xv{o}_{ci}",
                                     tag=f"xv{ci}")
                    xr = xts[ci]
                    src = bass.AP(xr.tensor, xr.offset,
                                  [xr.ap[0], [98, 4], [P, 2], [1, 64]])
                    nc.gpsimd.tensor_copy(xv, src)
                    xvs.append(xv)

                # QK projection: q tiles feature-major [128, OCT_W]; k tiles
                # slot-expanded [128, 4pair, 128] for contiguous stationaries.
                # ft order interleaves q/k so score matmuls unblock early.
                qs = [None] * 4
                ks = [None] * 4
                for ft in (0, 4, 1, 5, 2, 6, 3, 7):
                    ps = ps_head.tile([128, OCT_W], F32, name=f"qkp{o}_{ft}",
                                      tag="head")
                    if USE_FP8:
                        for ci in range(2):
                            nc.tensor.matmul(
                                ps, w8_sb[ci][:, :, 128 * ft:128 * (ft + 1)],
                                x8t[:, ci, :, 0:OCT_W],
                                start=(ci == 0),
                                stop=(ci == 1 and not has_bqk),
                                perf_mode=DR)
                    else:
                        for ci in range(4):
                            nc.tensor.matmul(
                                ps, wqk_sb[ci][:, 128 * ft:128 * (ft + 1)],
                                xts[ci], start=(ci == 0),
                                stop=(ci == 3 and not has_bqk))
                    if has_bqk:
                        nc.tensor.matmul(ps, bqk_sb[:, 128 * ft:128 * (ft + 1)],
                                         ones_row, start=False, stop=True)
                    if ft < 4:
                        q_sb = qk_pool.tile([128, OCT_TOK], BF, name=f"q{o}_{ft}",
                                            tag=f"q{ft}")
                        nc.scalar.copy(q_sb, ps[:, 0:OCT_TOK])
                        qs[ft] = q_sb
                    else:
                        k_sb = qk_pool.tile([128, 4, 128], BF, name=f"k{o}_{ft}",
                                            tag=f"k{ft}")
                        src = bass.AP(ps.tensor, ps.offset,
                                      [ps.ap[0], [98, 4], [P, 2], [1, 64]])
                        nc.vector.tensor_copy(k_sb, src)
                        ks[ft - 4] = k_sb

                # V projection (all pairs; slot-expanded stationary)
                v_sbs = []
                for p in range(4):
                    vps = ps_head.tile([128, C], F32, name=f"vp{o}_{p}",
                                       tag="head")
                    for ci in range(4):
                        nc.tensor.matmul(
                            vps, xvs[ci][:, p, :],
                            wv_sb[ci], start=(ci == 0),
                            stop=(ci == 3 and not has_bv))
                    if has_bv:
                        nc.tensor.matmul(vps, ones_row[:, 0:128], bv_sb,
                                         start=False, stop=True)
                    v_sb = v_pool.tile([128, NH, 33], BF, name=f"v{o}_{p}",
                                       tag="v")
                    vv = vps.rearrange("q (h d) -> q h d", h=NH)
                    nc.scalar.copy(v_sb[:, :, 0:32], vv)
                    v_sbs.append(v_sb)

                # scores: one matmul per (pair, head) covering both windows,
                # all 16 into one 4-bank PSUM tile (j selects the bank, so
                # each tile_position row group owns its own bank); row groups
                # rotate (j innermost) so LDWEIGHTS overlaps
                for p in range(4):
                    pt0 = 98 * p
                    stp = ps_st.tile([128, 4, 4, 128], F32, name=f"st{o}_{p}",
                                     tag="st")
                    sesp = se_pool.tile([128, 4, 4, 128], BF,
                                        name=f"se{o}_{p}", tag="se")
                    for i in range(4):
                        for j in range(4):
                            r = 32 * j
                            nc.tensor.matmul(
                                stp[:, j, i, 0:98],
                                ks[i][r:r + 32, p, :],
                                qs[i][r:r + 32, pt0:pt0 + 98],
                                start=True, stop=True,
                                tile_position=(r, 0))
                    nc.scalar.activation(
                        out=sesp[0:P, :, :, 0:P], in_=stp[0:P, :, :, 0:P],
                        func=Exp, scale=ESC)
                    nc.scalar.activation(
                        out=sesp[64:64 + P, :, :, 64:64 + P],
                        in_=stp[64:64 + P, :, :, P:2 * P],
                        func=Exp, scale=ESC)
                    # multiplicative rel-pos bias on the two diagonal
                    # blocks; j-halves so AV g=0 (j 0,1) releases early
                    for jh in range(2):
                        js = slice(2 * jh, 2 * jh + 2)
                        nc.gpsimd.tensor_mul(sesp[0:P, js, :, 0:P],
                                             sesp[0:P, js, :, 0:P],
                                             eb_sb[0:P, js, :, :])
                        nc.vector.tensor_mul(sesp[64:64 + P, js, :, 64:64 + P],
                                             sesp[64:64 + P, js, :, 64:64 + P],
                                             eb_sb[64:64 + P, js, :, :])

                    # AV: one matmul per head over the full 128-partition
                    # block-diagonal stationary; 33rd v column accumulates
                    # the softmax denominator
                    v_sb = v_sbs[p]
                    attn_sb = attn_pool.tile([128, NH, HD], BF,
                                             name=f"attn{o}_{p}", tag="attn")
                    for g in range(2):
                        # heads h = 4i + 2g + jj; one avt bank cycles
                        # av(g=0) -> av(g=1) -> atp within each pair
                        av = ps_avt.tile([128, 4, 2, 33], F32,
                                         name=f"av{o}_{p}_{g}", tag="avt")
                        for i in range(4):
                            for jj in range(2):
                                h = 4 * i + 2 * g + jj
                                nc.tensor.matmul(av[:, i, jj, :],
                                                 sesp[:, 2 * g + jj, i, :],
                                                 v_sb[:, h, :],
                                                 start=True, stop=True)
                        zr = zr_pool.tile([128, 4, 2], F32,
                                          name=f"zr{o}_{p}_{g}", tag="zr")
                        nc.vector.reciprocal(zr, av[:, :, :, 32])
                        attn_v = bass.AP(attn_sb.tensor, attn_sb.offset + 64 * g,
                                         [attn_sb.ap[0], [128, 4], [32, 2],
                                          [1, HD]])
                        nc.vector.tensor_mul(attn_v, av[:, :, :, 0:32],
                                             bcast_last(zr, HD))

                    # transpose to feature-major (same avt bank)
                    atp = ps_avt.tile([128, 4, 128], BF, name=f"atp{o}_{p}",
                                      tag="avt")
                    attn_flat = attn_sb.rearrange("q h d -> q (h d)")
                    for ci in range(4):
                        nc.tensor.transpose(atp[:, ci, :],
                                            attn_flat[:, 128 * ci:128 * (ci + 1)],
                                            ident)
                    if p == 0:
                        atT = atT_pool.tile([128, 4, 4, 128], BF,
                                            name=f"atT{o}", tag="atT")
                    dst = bass.AP(atT.tensor, atT.offset + 128 * p,
                                  [atT.ap[0], [512, 4], [1, 128]])
                    nc.vector.tensor_copy(dst, atp)

                # output projection, feature-major; the moving AP gathers the
                # 49-token windows out of the 64-token slots (dense 392 cols)
                osb = out_pool.tile([128, 4, OCT_TOK], BF, name=f"o{o}",
                                    tag="osb")
                for et in range(4):
                    poT = ps_po.tile([128, OCT_TOK], F32, name=f"po{o}_{et}",
                                     tag="po")
                    for ci in range(4):
                        rhs = bass.AP(atT.tensor, atT.offset + 512 * ci,
                                      [atT.ap[0], [128, 4], [64, 2], [1, P]])
                        nc.tensor.matmul(poT, wp_sb[ci][:, 128 * et:128 * (et + 1)],
                                         rhs, start=(ci == 0),
                                         stop=(ci == 3 and not has_bp))
                    if has_bp:
                        nc.tensor.matmul(poT, bpT_sb[:, 128 * et:128 * (et + 1)],
                                         ones_row[:, 0:OCT_TOK],
                                         start=False, stop=True)
                    if et < 2:
                        nc.scalar.copy(osb[:, et, :], poT)
                    else:
                        nc.vector.tensor_copy(osb[:, et, :], poT)
                # single store per octet on the Activation DGE ring
                outa = out[:, :]
                dst = bass.AP(outa.tensor, outa.offset + t0,
                              [[TOK, 128], [128 * TOK, 4], [1, OCT_TOK]])
                nc.scalar.dma_start(out=dst, in_=osb)
    return nc


def _host_prep(x, qkv_w, qkv_b, proj_w, proj_b, rpb_table, rel_index):
    scale = HD ** -0.5
    # weights: qkv feature order is (3, NH, HD) -> q=0:512, k=512:1024, v=1024:1536
    wq = qkv_w[0:C, :] * scale          # fold attention scale into q
    wk = qkv_w[C:2 * C, :]
    wv = qkv_w[2 * C:3 * C, :]
    wv_t = np.ascontiguousarray(wv.T).astype(BF16)              # [C, C]
    wp_t = np.ascontiguousarray(proj_w.T).astype(BF16)          # [C, C]

    bq = qkv_b[0:C] * scale
    bk = qkv_b[C:2 * C]
    if USE_FP8:
        wqk = np.concatenate([wq.T * SQ, wk.T * SK], axis=1)    # [C, 2C]
        # feature f = ci*256 + i*128 + p  ->  w8[ci][p, i, m]
        w8 = np.ascontiguousarray(
            wqk.reshape(2, 2, 128, 2 * C).transpose(0, 2, 1, 3)).astype(FP8)
        bqk = np.concatenate([bq * SQ, bk * SK])[None, :].astype(BF16)
    else:
        wqk = np.concatenate([wq.T, wk.T], axis=1).astype(BF16)
        bqk = np.concatenate([bq, bk])[None, :].astype(BF16)

    bias = rpb_table[rel_index]                  # [n, m, NH]
    biasT = np.transpose(bias, (2, 1, 0))        # [h, m, n]
    ebias = np.exp(biasT.astype(np.float64)).astype(np.float32)
    ebp = np.ones((128, NH, P), np.float32)
    ebp[0:P] = np.transpose(ebias, (1, 0, 2))    # rows 0:49  (window slot 0)
    ebp[64:64 + P] = ebp[0:P]                    # rows 64:113 (window slot 1)
    # regroup heads: [128, j, i, P] with head 4*i + j at [:, j, i, :]
    eb = np.ascontiguousarray(
        ebp.reshape(128, 4, 4, P).transpose(0, 2, 1, 3)).astype(BF16)

    bv_ = qkv_b[2 * C:3 * C]
    bv = bv_[None, :].astype(BF16)
    bp = proj_b[None, :].astype(BF16)

    has_bqk = bool(np.any(qkv_b[0:2 * C] != 0))
    has_bv = bool(np.any(bv_ != 0))
    has_bp = bool(np.any(proj_b != 0))

    in_maps = []
    for c in range(NCORES):
        xc = np.asarray(x[c * B_CORE:(c + 1) * B_CORE]).reshape(TOK, C)
        xTc = np.zeros((C, TOK_PAD), np.float32)
        xTc[:, :TOK] = xc.T
        m = {"xT": xTc.astype(BF16), "wv": wv_t, "wp": wp_t, "eb": eb}
        if USE_FP8:
            x8c = np.zeros((C, TOK_PAD8), np.float32)
            x8c[:, :TOK] = xc.T
            m["x8"] = np.ascontiguousarray(
                x8c.reshape(2, 2, 128, TOK_PAD8).transpose(2, 0, 1, 3)).astype(FP8)
            m["w8"] = w8
        else:
            m["wqk"] = wqk
        if has_bqk:
            m["bqk"] = bqk
        if has_bv:
            m["bv"] = bv
        if has_bp:
            m["bp"] = bp
        in_maps.append(m)
    return in_maps, has_bqk, has_bv, has_bp


def kernel(x, qkv_w, qkv_b, proj_w, proj_b, rpb_table, rel_index):
    from concourse import bacc
    from concourse.bass_utils import run_bass_kernel_spmd

    in_maps, has_bqk, has_bv, has_bp = _host_prep(
        np.asarray(x, np.float32), np.asarray(qkv_w, np.float32),
        np.asarray(qkv_b, np.float32), np.asarray(proj_w, np.float32),
        np.asarray(proj_b, np.float32), np.asarray(rpb_table, np.float32),
        np.asarray(rel_index))

    nc = bacc.Bacc()
    _build(nc, has_bqk, has_bv, has_bp)
    nc.finalize()

    trace = os.environ.get("BASS_KERNEL_TRACE", "") == "1"
    res = run_bass_kernel_spmd(nc, in_maps, core_ids=list(range(NCORES)),
                               trace=trace)
    if trace and res.exec_time_ns is not None:
        print(f"HW exec time: {res.exec_time_ns} ns", flush=True)

    outs = [r["out"].astype(np.float32).T.reshape(B_CORE, P, C)
            for r in res.results]
    return np.concatenate(outs, axis=0)


# revision 3
# speedup vs baseline: 1.1700x; 1.0177x over previous
"""Swin windowed attention (B_=2048 windows, N=49 tokens, C=512, 16 heads)
on 8 Trainium2 NeuronCores, data-parallel over windows (256 windows/core).

Per-core layout (v3; ~1.4x fewer PE instructions and ~35%% less engine work
than the v1 baseline, with input/output DMA rings decoupled):

  - q,k projection runs in fp8e4 with DoubleRow perf mode (K=256/pass, 2
    passes): 16 matmuls/octet instead of 32.  Host folds the attention
    scale and power-of-2 fp8 scales (q x256, k x64) into the weights; the
    combined descale rides the exp activation's scale operand for free.
  - scores^T are computed two-windows-per-matmul: the stationary is a
    [32, 128] slice of a "kslot" tile whose columns are the pair's two
    windows at 64-aligned slots (materialized contiguously so FWL stays
    on); the moving q covers both windows (98 cols).  64 score matmuls
    per octet at 4 rotating row groups (LDWEIGHTS overlaps), all 16 of a
    pair accumulating into one 4-bank PSUM tile (bank = tile_position row).
  - exp'd scores land in zero-padded block-diagonal "sesp" tiles
    [128, 4j, 4i, 128] (w0 at [0:49, ..., 0:49], w1 at [64:113, ..., 64:113],
    zeros memset once per pool buffer).  One AV matmul per (pair, head)
    contracts all 128 partitions with the contiguous [128, 128] stationary:
    64 AV matmuls/octet; a 33rd ones-column of v accumulates the softmax
    denominator.
  - v projection uses slot-expanded x copies produced on the otherwise-idle
    GPSIMD engine (SBUF->SBUF; PSUM is GPSIMD-inaccessible, and matmul
    weights APs only allow one free dim).
  - normalization = DVE reciprocal + free-broadcast multiply; attention
    output is PE-transposed to feature-major, and the output projection is
    emitted feature-major with the moving AP gathering 49-token windows out
    of the 64-token slots; out^T [C, TOK] is un-transposed on the host.
  - engine budget: ACT exp/q/v/osb, DVE kslot/eb-w1/recip/attn/atT,
    GPSIMD xv/eb-w0/memsets.  PSUM: 2 banks QK+V, 4 banks scores, 1 bank
    AV+transpose, 1 bank projection - the head of octet o+1 shares nothing
    with the tail of octet o.  Inputs ride the SP DGE ring (one fused DMA
    per tensor per octet), the single output DMA rides the ACT ring.
"""

import os
import sys

import numpy as np
import ml_dtypes

if "/opt/trn_rl_repo" not in sys.path:
    sys.path.insert(0, "/opt/trn_rl_repo")

P = 49          # tokens per window
NH = 16         # heads
HD = 32         # head dim
C = 512         # model dim
NCORES = 8
B_TOTAL = 2048
B_CORE = B_TOTAL // NCORES        # 256 windows per core
TOK = B_CORE * P                  # 12544 tokens per core
OCT = 32                          # octets (8 windows) per core
OCT_TOK = 8 * P                   # 392 tokens per octet
OCT_W = OCT_TOK + 16              # octet tile width incl 16-token overlap
OCT_W8 = OCT_TOK + 24             # fp8 tile width (dim-1 step must be %16)
TOK_PAD = TOK + 16
TOK_PAD8 = TOK + 32
BF16 = ml_dtypes.bfloat16
FP8 = ml_dtypes.float8_e4m3fn

USE_FP8 = True
SQ = 256.0     # fp8 scale folded into q weights (power of 2)
SK = 64.0      # fp8 scale folded into k weights
ESC = 1.0 / (SQ * SK) if USE_FP8 else 1.0


def _build(nc, has_bqk, has_bv, has_bp, n_oct=OCT):
    import concourse.bass as bass
    import concourse.mybir as mybir
    from concourse.tile import TileContext
    from concourse.masks import make_identity

    F32 = mybir.dt.float32
    BF = mybir.dt.bfloat16
    F8 = mybir.dt.float8e4
    Exp = mybir.ActivationFunctionType.Exp
    DR = mybir.MatmulPerfMode.DoubleRow

    if USE_FP8:
        x8 = nc.dram_tensor("x8", [128, 2, 2, TOK_PAD8], F8, kind="ExternalInput")
        w8 = nc.dram_tensor("w8", [2, 128, 2, 2 * C], F8, kind="ExternalInput")
    else:
        wqk = nc.dram_tensor("wqk", [C, 2 * C], BF, kind="ExternalInput")
    xT = nc.dram_tensor("xT", [C, TOK_PAD], BF, kind="ExternalInput")
    wv = nc.dram_tensor("wv", [C, C], BF, kind="ExternalInput")
    wp = nc.dram_tensor("wp", [C, C], BF, kind="ExternalInput")
    eb = nc.dram_tensor("eb", [128, 4, 4, P], BF, kind="ExternalInput")
    bqk = bv = bp = None
    if has_bqk:
        bqk = nc.dram_tensor("bqk", [1, 2 * C], BF, kind="ExternalInput")
    if has_bv:
        bv = nc.dram_tensor("bv", [1, C], BF, kind="ExternalInput")
    if has_bp:
        bp = nc.dram_tensor("bp", [1, C], BF, kind="ExternalInput")
    out = nc.dram_tensor("out", [C, TOK], BF, kind="ExternalOutput")

    def bcast_last(ap, n):
        return bass.AP(ap.tensor, ap.offset, [*ap.ap, [0, n]])

    with TileContext(nc) as tc:
        with (
            tc.tile_pool(name="singles", bufs=1) as singles,
            tc.tile_pool(name="xin", bufs=3) as x_pool,
            tc.tile_pool(name="qk", bufs=2) as qk_pool,
            tc.tile_pool(name="vsb", bufs=6) as v_pool,
            tc.tile_pool(name="se", bufs=3) as se_pool,
            tc.tile_pool(name="zr", bufs=8) as zr_pool,
            tc.tile_pool(name="attn", bufs=6) as attn_pool,
            tc.tile_pool(name="att", bufs=3) as atT_pool,
            tc.tile_pool(name="osb", bufs=4) as out_pool,
            tc.tile_pool(name="ps_head", bufs=2, space="PSUM") as ps_head,
            tc.tile_pool(name="ps_st", bufs=1, space="PSUM") as ps_st,
            tc.tile_pool(name="ps_avt", bufs=1, space="PSUM") as ps_avt,
            tc.tile_pool(name="ps_po", bufs=1, space="PSUM") as ps_po,
        ):
            # --- constants / weights ---
            if USE_FP8:
                w8_sb = []
                for ci in range(2):
                    w8_t = singles.tile([128, 2, 2 * C], F8, name=f"w8{ci}")
                    nc.sync.dma_start(out=w8_t, in_=w8[ci, :, :, :])
                    w8_sb.append(w8_t)
            else:
                wqk_sb = []
                for ci in range(4):
                    wqk_t = singles.tile([128, 2 * C], BF, name=f"wqk{ci}")
                    nc.sync.dma_start(out=wqk_t, in_=wqk[128 * ci:128 * (ci + 1), :])
                    wqk_sb.append(wqk_t)
            wv_sb = []
            wp_sb = []
            for ci in range(4):
                wv_t = singles.tile([128, C], BF, name=f"wv{ci}")
                nc.sync.dma_start(out=wv_t, in_=wv[128 * ci:128 * (ci + 1), :])
                wv_sb.append(wv_t)
                wp_t = singles.tile([128, C], BF, name=f"wp{ci}")
                nc.sync.dma_start(out=wp_t, in_=wp[128 * ci:128 * (ci + 1), :])
                wp_sb.append(wp_t)
            eb_sb = singles.tile([128, 4, 4, P], BF, name="ebsb")
            nc.sync.dma_start(out=eb_sb, in_=eb[:, :, :, :])
            ident = singles.tile([128, 128], BF, name="ident")
            make_identity(nc, ident)
            bqk_sb = bv_sb = bpT_sb = ones_row = None
            if has_bqk or has_bv or has_bp:
                ones_row = singles.tile([1, OCT_W], BF, name="onesrow")
                nc.vector.memset(ones_row, 1.0)
            if has_bqk:
                bqk_sb = singles.tile([1, 2 * C], BF, name="bqksb")
                nc.sync.dma_start(out=bqk_sb, in_=bqk[:, :])
            if has_bv:
                bv_sb = singles.tile([1, C], BF, name="bvsb")
                nc.sync.dma_start(out=bv_sb, in_=bv[:, :])
            if has_bp:
                bpT_sb = singles.tile([1, C], BF, name="bpsb")
                nc.sync.dma_start(out=bpT_sb, in_=bp[:, :])

            # pre-zero the block-diagonal ses tiles (exp/eb rewrite only the
            # two diagonal blocks; the zero padding is what makes the fused
            # 2-window AV stationary sound) and pre-set the v denominator
            # ones column (the v copy writes only [:, :, 0:32]).
            for b in range(3):
                sez = se_pool.tile([128, 4, 4, 128], BF, name=f"sez{b}",
                                   tag="se")
                nc.gpsimd.memset(sez, 0.0)
            for b in range(6):
                vz = v_pool.tile([128, NH, 33], BF, name=f"vz{b}", tag="v")
                nc.gpsimd.memset(vz, 1.0)

            # --- main loop ---
            for o in range(n_oct):
                t0 = o * OCT_TOK
                # input DMAs (SP ring only carries inputs; outputs go via
                # the Activation ring so next-octet loads are never queued
                # behind this octet's stores)
                if USE_FP8:
                    x8t = x_pool.tile([128, 2, 2, OCT_W8], F8,
                                      name=f"x8{o}", tag="x8")
                    x8a = x8[:, :, :, :]
                    src8 = bass.AP(
                        x8a.tensor, x8a.offset + t0,
                        [[4 * TOK_PAD8, 128], [TOK_PAD8, 4], [1, OCT_W8]])
                    nc.sync.dma_start(out=x8t, in_=src8)
                xtile = x_pool.tile([128, 4, OCT_W], BF, name=f"xt{o}",
                                    tag="xt")
                xTa = xT[:, :]
                srcx = bass.AP(
                    xTa.tensor, xTa.offset + t0,
                    [[TOK_PAD, 128], [128 * TOK_PAD, 4], [1, OCT_W]])
                nc.sync.dma_start(out=xtile, in_=srcx)
                xts = [xtile[:, ci, :] for ci in range(4)]
                # slot-expanded copies for the V stationary (weights APs may
                # only have one free dim); SBUF->SBUF so Pool can do them
                xvs = []
                for ci in range(4):
                    xv = x_pool.tile([128, 4, 128], BF, name=f"xv{o}_{ci}",
                                     tag=f"xv{ci}")
                    xr = xts[ci]
                    src = bass.AP(xr.tensor, xr.offset,
                                  [xr.ap[0], [98, 4], [P, 2], [1, 64]])
                    nc.gpsimd.tensor_copy(xv, src)
                    xvs.append(xv)

                # QK projection: q tiles feature-major [128, OCT_W]; k tiles
                # slot-expanded [128, 4pair, 128] for contiguous stationaries.
                # ft order interleaves q/k so score matmuls unblock early.
                qs = [None] * 4
                ks = [None] * 4
                for ft in (0, 4, 1, 5, 2, 6, 3, 7):
                    ps = ps_head.tile([128, OCT_W], F32, name=f"qkp{o}_{ft}",
                                      tag="head")
                    if USE_FP8:
                        for ci in range(2):
                            nc.tensor.matmul(
                                ps, w8_sb[ci][:, :, 128 * ft:128 * (ft + 1)],
                                x8t[:, ci, :, 0:OCT_W],
                                start=(ci == 0),
                                stop=(ci == 1 and not has_bqk),
                                perf_mode=DR)
                    else:
                        for ci in range(4):
                            nc.tensor.matmul(
                                ps, wqk_sb[ci][:, 128 * ft:128 * (ft + 1)],
                                xts[ci], start=(ci == 0),
                                stop=(ci == 3 and not has_bqk))
                    if has_bqk:
                        nc.tensor.matmul(ps, bqk_sb[:, 128 * ft:128 * (ft + 1)],
                                         ones_row, start=False, stop=True)
                    if ft < 4:
                        q_sb = qk_pool.tile([128, OCT_TOK], BF, name=f"q{o}_{ft}",
                                            tag=f"q{ft}")
                        nc.scalar.copy(q_sb, ps[:, 0:OCT_TOK])
                        qs[ft] = q_sb
                    else:
                        k_sb = qk_pool.tile([128, 4, 128], BF, name=f"k{o}_{ft}",
                                            tag=f"k{ft}")
                        src = bass.AP(ps.tensor, ps.offset,
                                      [ps.ap[0], [98, 4], [P, 2], [1, 64]])
                        nc.vector.tensor_copy(k_sb, src)
                        ks[ft - 4] = k_sb

                # V projection (all pairs; slot-expanded stationary)
                v_sbs = []
                for p in range(4):
                    vps = ps_head.tile([128, C], F32, name=f"vp{o}_{p}",
                                       tag="head")
                    for ci in range(4):
                        nc.tensor.matmul(
                            vps, xvs[ci][:, p, :],
                            wv_sb[ci], start=(ci == 0),
                            stop=(ci == 3 and not has_bv))
                    if has_bv:
                        nc.tensor.matmul(vps, ones_row[:, 0:128], bv_sb,
                                         start=False, stop=True)
                    v_sb = v_pool.tile([128, NH, 33], BF, name=f"v{o}_{p}",
                                       tag="v")
                    vv = vps.rearrange("q (h d) -> q h d", h=NH)
                    nc.scalar.copy(v_sb[:, :, 0:32], vv)
                    v_sbs.append(v_sb)

                # scores: one matmul per (pair, head) covering both windows,
                # all 16 into one 4-bank PSUM tile (j selects the bank, so
                # each tile_position row group owns its own bank); row groups
                # rotate (j innermost) so LDWEIGHTS overlaps
                for p in range(4):
                    pt0 = 98 * p
                    stp = ps_st.tile([128, 4, 4, 128], F32, name=f"st{o}_{p}",
                                     tag="st")
                    sesp = se_pool.tile([128, 4, 4, 128], BF,
                                        name=f"se{o}_{p}", tag="se")
                    for i in range(4):
                        for j in range(4):
                            r = 32 * j
                            nc.tensor.matmul(
                                stp[:, j, i, 0:98],
                                ks[i][r:r + 32, p, :],
                                qs[i][r:r + 32, pt0:pt0 + 98],
                                start=True, stop=True,
                                tile_position=(r, 0))
                    with tc.high_priority():
                        nc.scalar.activation(
                            out=sesp[0:P, :, :, 0:P], in_=stp[0:P, :, :, 0:P],
                            func=Exp, scale=ESC)
                        nc.scalar.activation(
                            out=sesp[64:64 + P, :, :, 64:64 + P],
                            in_=stp[64:64 + P, :, :, P:2 * P],
                            func=Exp, scale=ESC)
                    # multiplicative rel-pos bias on the two diagonal
                    # blocks; j-halves so AV g=0 (j 0,1) releases early
                    for jh in range(2):
                        js = slice(2 * jh, 2 * jh + 2)
                        nc.gpsimd.tensor_mul(sesp[0:P, js, :, 0:P],
                                             sesp[0:P, js, :, 0:P],
                                             eb_sb[0:P, js, :, :])
                        nc.vector.tensor_mul(sesp[64:64 + P, js, :, 64:64 + P],
                                             sesp[64:64 + P, js, :, 64:64 + P],
                                             eb_sb[64:64 + P, js, :, :])

                    # AV: one matmul per head over the full 128-partition
                    # block-diagonal stationary; 33rd v column accumulates
                    # the softmax denominator
                    v_sb = v_sbs[p]
                    attn_sb = attn_pool.tile([128, NH, HD], BF,
                                             name=f"attn{o}_{p}", tag="attn")
                    for g in range(2):
                        # heads h = 4i + 2g + jj; one avt bank cycles
                        # av(g=0) -> av(g=1) -> atp within each pair
                        av = ps_avt.tile([128, 4, 2, 33], F32,
                                         name=f"av{o}_{p}_{g}", tag="avt")
                        for i in range(4):
                            for jj in range(2):
                                h = 4 * i + 2 * g + jj
                                nc.tensor.matmul(av[:, i, jj, :],
                                                 sesp[:, 2 * g + jj, i, :],
                                                 v_sb[:, h, :],
                                                 start=True, stop=True)
                        zr = zr_pool.tile([128, 4, 2], F32,
                                          name=f"zr{o}_{p}_{g}", tag="zr")
                        with tc.high_priority():
                            nc.vector.reciprocal(zr, av[:, :, :, 32])
                            attn_v = bass.AP(attn_sb.tensor,
                                             attn_sb.offset + 64 * g,
                                             [attn_sb.ap[0], [128, 4], [32, 2],
                                              [1, HD]])
                            nc.vector.tensor_mul(attn_v, av[:, :, :, 0:32],
                                                 bcast_last(zr, HD))

                    # transpose to feature-major (same avt bank)
                    atp = ps_avt.tile([128, 4, 128], BF, name=f"atp{o}_{p}",
                                      tag="avt")
                    attn_flat = attn_sb.rearrange("q h d -> q (h d)")
                    for ci in range(4):
                        nc.tensor.transpose(atp[:, ci, :],
                                            attn_flat[:, 128 * ci:128 * (ci + 1)],
                                            ident)
                    if p == 0:
                        atT = atT_pool.tile([128, 4, 4, 128], BF,
                                            name=f"atT{o}", tag="atT")
                    dst = bass.AP(atT.tensor, atT.offset + 128 * p,
                                  [atT.ap[0], [512, 4], [1, 128]])
                    with tc.high_priority():
                        nc.vector.tensor_copy(dst, atp)

                # output projection, feature-major; the moving AP gathers the
                # 49-token windows out of the 64-token slots (dense 392 cols)
                osb = out_pool.tile([128, 4, OCT_TOK], BF, name=f"o{o}",
                                    tag="osb")
                for et in range(4):
                    poT = ps_po.tile([128, OCT_TOK], F32, name=f"po{o}_{et}",
                                     tag="po")
                    for ci in range(4):
                        rhs = bass.AP(atT.tensor, atT.offset + 512 * ci,
                                      [atT.ap[0], [128, 4], [64, 2], [1, P]])
                        nc.tensor.matmul(poT, wp_sb[ci][:, 128 * et:128 * (et + 1)],
                                         rhs, start=(ci == 0),
                                         stop=(ci == 3 and not has_bp))
                    if has_bp:
                        nc.tensor.matmul(poT, bpT_sb[:, 128 * et:128 * (et + 1)],
                                         ones_row[:, 0:OCT_TOK],
                                         start=False, stop=True)
                    if et < 2:
                        nc.scalar.copy(osb[:, et, :], poT)
                    else:
                        nc.vector.tensor_copy(osb[:, et, :], poT)
                # single store per octet on the Activation DGE ring
                outa = out[:, :]
                dst = bass.AP(outa.tensor, outa.offset + t0,
                              [[TOK, 128], [128 * TOK, 4], [1, OCT_TOK]])
                nc.scalar.dma_start(out=dst, in_=osb)
    return nc


def _host_prep(x, qkv_w, qkv_b, proj_w, proj_b, rpb_table, rel_index):
    scale = HD ** -0.5
    # weights: qkv feature order is (3, NH, HD) -> q=0:512, k=512:1024, v=1024:1536
    wq = qkv_w[0:C, :] * scale          # fold attention scale into q
    wk = qkv_w[C:2 * C, :]
    wv = qkv_w[2 * C:3 * C, :]
    wv_t = np.ascontiguousarray(wv.T).astype(BF16)              # [C, C]
    wp_t = np.ascontiguousarray(proj_w.T).astype(BF16)          # [C, C]

    bq = qkv_b[0:C] * scale
    bk = qkv_b[C:2 * C]
    if USE_FP8:
        wqk = np.concatenate([wq.T * SQ, wk.T * SK], axis=1)    # [C, 2C]
        # feature f = ci*256 + i*128 + p  ->  w8[ci][p, i, m]
        w8 = np.ascontiguousarray(
            wqk.reshape(2, 2, 128, 2 * C).transpose(0, 2, 1, 3)).astype(FP8)
        bqk = np.concatenate([bq * SQ, bk * SK])[None, :].astype(BF16)
    else:
        wqk = np.concatenate([wq.T, wk.T], axis=1).astype(BF16)
        bqk = np.concatenate([bq, bk])[None, :].astype(BF16)

    bias = rpb_table[rel_index]                  # [n, m, NH]
    biasT = np.transpose(bias, (2, 1, 0))        # [h, m, n]
    ebias = np.exp(biasT.astype(np.float64)).astype(np.float32)
    ebp = np.ones((128, NH, P), np.float32)
    ebp[0:P] = np.transpose(ebias, (1, 0, 2))    # rows 0:49  (window slot 0)
    ebp[64:64 + P] = ebp[0:P]                    # rows 64:113 (window slot 1)
    # regroup heads: [128, j, i, P] with head 4*i + j at [:, j, i, :]
    eb = np.ascontiguousarray(
        ebp.reshape(128, 4, 4, P).transpose(0, 2, 1, 3)).astype(BF16)

    bv_ = qkv_b[2 * C:3 * C]
    bv = bv_[None, :].astype(BF16)
    bp = proj_b[None, :].astype(BF16)

    has_bqk = bool(np.any(qkv_b[0:2 * C] != 0))
    has_bv = bool(np.any(bv_ != 0))
    has_bp = bool(np.any(proj_b != 0))

    in_maps = []
    for c in range(NCORES):
        xc = np.asarray(x[c * B_CORE:(c + 1) * B_CORE]).reshape(TOK, C)
        xTc = np.zeros((C, TOK_PAD), np.float32)
        xTc[:, :TOK] = xc.T
        m = {"xT": xTc.astype(BF16), "wv": wv_t, "wp": wp_t, "eb": eb}
        if USE_FP8:
            x8c = np.zeros((C, TOK_PAD8), np.float32)
            x8c[:, :TOK] = xc.T
            m["x8"] = np.ascontiguousarray(
                x8c.reshape(2, 2, 128, TOK_PAD8).transpose(2, 0, 1, 3)).astype(FP8)
            m["w8"] = w8
        else:
            m["wqk"] = wqk
        if has_bqk:
            m["bqk"] = bqk
        if has_bv:
            m["bv"] = bv
        if has_bp:
            m["bp"] = bp
        in_maps.append(m)
    return in_maps, has_bqk, has_bv, has_bp


def kernel(x, qkv_w, qkv_b, proj_w, proj_b, rpb_table, rel_index):
    from concourse import bacc
    from concourse.bass_utils import run_bass_kernel_spmd

    in_maps, has_bqk, has_bv, has_bp = _host_prep(
        np.asarray(x, np.float32), np.asarray(qkv_w, np.float32),
        np.asarray(qkv_b, np.float32), np.asarray(proj_w, np.float32),
        np.asarray(proj_b, np.float32), np.asarray(rpb_table, np.float32),
        np.asarray(rel_index))

    nc = bacc.Bacc()
    _build(nc, has_bqk, has_bv, has_bp)
    nc.finalize()

    trace = os.environ.get("BASS_KERNEL_TRACE", "") == "1"
    res = run_bass_kernel_spmd(nc, in_maps, core_ids=list(range(NCORES)),
                               trace=trace)
    if trace and res.exec_time_ns is not None:
        print(f"HW exec time: {res.exec_time_ns} ns", flush=True)

    outs = [r["out"].astype(np.float32).T.reshape(B_CORE, P, C)
            for r in res.results]
    return np.concatenate(outs, axis=0)


# revision 4
# speedup vs baseline: 1.1811x; 1.0095x over previous
"""Swin windowed attention (B_=2048 windows, N=49 tokens, C=512, 16 heads)
on 8 Trainium2 NeuronCores, data-parallel over windows (256 windows/core).

Per-core layout (v3; ~1.4x fewer PE instructions and ~35%% less engine work
than the v1 baseline, with input/output DMA rings decoupled):

  - q,k projection runs in fp8e4 with DoubleRow perf mode (K=256/pass, 2
    passes): 16 matmuls/octet instead of 32.  Host folds the attention
    scale and power-of-2 fp8 scales (q x256, k x64) into the weights; the
    combined descale rides the exp activation's scale operand for free.
  - scores^T are computed two-windows-per-matmul: the stationary is a
    [32, 128] slice of a "kslot" tile whose columns are the pair's two
    windows at 64-aligned slots (materialized contiguously so FWL stays
    on); the moving q covers both windows (98 cols).  64 score matmuls
    per octet at 4 rotating row groups (LDWEIGHTS overlaps), all 16 of a
    pair accumulating into one 4-bank PSUM tile (bank = tile_position row).
  - exp'd scores land in zero-padded block-diagonal "sesp" tiles
    [128, 4j, 4i, 128] (w0 at [0:49, ..., 0:49], w1 at [64:113, ..., 64:113],
    zeros memset once per pool buffer).  One AV matmul per (pair, head)
    contracts all 128 partitions with the contiguous [128, 128] stationary:
    64 AV matmuls/octet; a 33rd ones-column of v accumulates the softmax
    denominator.
  - v projection uses slot-expanded x copies produced on the otherwise-idle
    GPSIMD engine (SBUF->SBUF; PSUM is GPSIMD-inaccessible, and matmul
    weights APs only allow one free dim).
  - normalization = DVE reciprocal + free-broadcast multiply; attention
    output is PE-transposed to feature-major, and the output projection is
    emitted feature-major with the moving AP gathering 49-token windows out
    of the 64-token slots; out^T [C, TOK] is un-transposed on the host.
  - engine budget: ACT exp/q/v/osb, DVE kslot/eb-w1/recip/attn/atT,
    GPSIMD xv/eb-w0/memsets.  PSUM: 2 banks QK+V, 4 banks scores, 1 bank
    AV+transpose, 1 bank projection - the head of octet o+1 shares nothing
    with the tail of octet o.  Inputs ride the SP DGE ring (one fused DMA
    per tensor per octet), the single output DMA rides the ACT ring.
"""

import os
import sys

import numpy as np
import ml_dtypes

if "/opt/trn_rl_repo" not in sys.path:
    sys.path.insert(0, "/opt/trn_rl_repo")

P = 49          # tokens per window
NH = 16         # heads
HD = 32         # head dim
C = 512         # model dim
NCORES = 8
B_TOTAL = 2048
B_CORE = B_TOTAL // NCORES        # 256 windows per core
TOK = B_CORE * P                  # 12544 tokens per core
OCT = 32                          # octets (8 windows) per core
OCT_TOK = 8 * P                   # 392 tokens per octet
OCT_W = OCT_TOK + 16              # octet tile width incl 16-token overlap
OCT_W8 = OCT_TOK + 24             # fp8 tile width (dim-1 step must be %16)
TOK_PAD = TOK + 16
TOK_PAD8 = TOK + 32
BF16 = ml_dtypes.bfloat16
FP8 = ml_dtypes.float8_e4m3fn

USE_FP8 = True
SQ = 256.0     # fp8 scale folded into q weights (power of 2)
SK = 64.0      # fp8 scale folded into k weights
ESC = 1.0 / (SQ * SK) if USE_FP8 else 1.0


def _build(nc, has_bqk, has_bv, has_bp, n_oct=OCT):
    import concourse.bass as bass
    import concourse.mybir as mybir
    from concourse.tile import TileContext
    from concourse.masks import make_identity

    F32 = mybir.dt.float32
    BF = mybir.dt.bfloat16
    F8 = mybir.dt.float8e4
    Exp = mybir.ActivationFunctionType.Exp
    DR = mybir.MatmulPerfMode.DoubleRow

    if USE_FP8:
        x8 = nc.dram_tensor("x8", [128, 2, 2, TOK_PAD8], F8, kind="ExternalInput")
        w8 = nc.dram_tensor("w8", [2, 128, 2, 2 * C], F8, kind="ExternalInput")
    else:
        wqk = nc.dram_tensor("wqk", [C, 2 * C], BF, kind="ExternalInput")
    xT = nc.dram_tensor("xT", [C, TOK_PAD], BF, kind="ExternalInput")
    wv = nc.dram_tensor("wv", [C, C], BF, kind="ExternalInput")
    wp = nc.dram_tensor("wp", [C, C], BF, kind="ExternalInput")
    eb = nc.dram_tensor("eb", [128, 4, 4, P], BF, kind="ExternalInput")
    bqk = bv = bp = None
    if has_bqk:
        bqk = nc.dram_tensor("bqk", [1, 2 * C], BF, kind="ExternalInput")
    if has_bv:
        bv = nc.dram_tensor("bv", [1, C], BF, kind="ExternalInput")
    if has_bp:
        bp = nc.dram_tensor("bp", [1, C], BF, kind="ExternalInput")
    out = nc.dram_tensor("out", [C, TOK], BF, kind="ExternalOutput")

    def bcast_last(ap, n):
        return bass.AP(ap.tensor, ap.offset, [*ap.ap, [0, n]])

    with TileContext(nc) as tc:
        with (
            tc.tile_pool(name="singles", bufs=1) as singles,
            tc.tile_pool(name="xin", bufs=3) as x_pool,
            tc.tile_pool(name="qk", bufs=2) as qk_pool,
            tc.tile_pool(name="vsb", bufs=6) as v_pool,
            tc.tile_pool(name="se", bufs=3) as se_pool,
            tc.tile_pool(name="zr", bufs=8) as zr_pool,
            tc.tile_pool(name="attn", bufs=6) as attn_pool,
            tc.tile_pool(name="att", bufs=3) as atT_pool,
            tc.tile_pool(name="osb", bufs=4) as out_pool,
            tc.tile_pool(name="ps_head", bufs=2, space="PSUM") as ps_head,
            tc.tile_pool(name="ps_st", bufs=1, space="PSUM") as ps_st,
            tc.tile_pool(name="ps_avt", bufs=1, space="PSUM") as ps_avt,
            tc.tile_pool(name="ps_po", bufs=1, space="PSUM") as ps_po,
        ):
            # --- constants / weights ---
            if USE_FP8:
                w8_sb = []
                for ci in range(2):
                    w8_t = singles.tile([128, 2, 2 * C], F8, name=f"w8{ci}")
                    nc.sync.dma_start(out=w8_t, in_=w8[ci, :, :, :])
                    w8_sb.append(w8_t)
            else:
                wqk_sb = []
                for ci in range(4):
                    wqk_t = singles.tile([128, 2 * C], BF, name=f"wqk{ci}")
                    nc.sync.dma_start(out=wqk_t, in_=wqk[128 * ci:128 * (ci + 1), :])
                    wqk_sb.append(wqk_t)
            wv_sb = []
            wp_sb = []
            for ci in range(4):
                wv_t = singles.tile([128, C], BF, name=f"wv{ci}")
                nc.sync.dma_start(out=wv_t, in_=wv[128 * ci:128 * (ci + 1), :])
                wv_sb.append(wv_t)
                wp_t = singles.tile([128, C], BF, name=f"wp{ci}")
                nc.sync.dma_start(out=wp_t, in_=wp[128 * ci:128 * (ci + 1), :])
                wp_sb.append(wp_t)
            eb_sb = singles.tile([128, 4, 4, P], BF, name="ebsb")
            nc.sync.dma_start(out=eb_sb, in_=eb[:, :, :, :])
            ident = singles.tile([128, 128], BF, name="ident")
            make_identity(nc, ident)
            bqk_sb = bv_sb = bpT_sb = ones_row = None
            if has_bqk or has_bv or has_bp:
                ones_row = singles.tile([1, OCT_W], BF, name="onesrow")
                nc.vector.memset(ones_row, 1.0)
            if has_bqk:
                bqk_sb = singles.tile([1, 2 * C], BF, name="bqksb")
                nc.sync.dma_start(out=bqk_sb, in_=bqk[:, :])
            if has_bv:
                bv_sb = singles.tile([1, C], BF, name="bvsb")
                nc.sync.dma_start(out=bv_sb, in_=bv[:, :])
            if has_bp:
                bpT_sb = singles.tile([1, C], BF, name="bpsb")
                nc.sync.dma_start(out=bpT_sb, in_=bp[:, :])

            # pre-zero the block-diagonal ses tiles (exp/eb rewrite only the
            # two diagonal blocks; the zero padding is what makes the fused
            # 2-window AV stationary sound) and pre-set the v denominator
            # ones column (the v copy writes only [:, :, 0:32]).
            for b in range(3):
                sez = se_pool.tile([128, 4, 4, 128], BF, name=f"sez{b}",
                                   tag="se")
                nc.gpsimd.memset(sez, 0.0)
            for b in range(6):
                vz = v_pool.tile([128, NH, 33], BF, name=f"vz{b}", tag="v")
                nc.gpsimd.memset(vz, 1.0)

            # --- main loop ---
            for o in range(n_oct):
                t0 = o * OCT_TOK
                # input DMAs (SP ring only carries inputs; outputs go via
                # the Activation ring so next-octet loads are never queued
                # behind this octet's stores)
                if USE_FP8:
                    x8t = x_pool.tile([128, 2, 2, OCT_W8], F8,
                                      name=f"x8{o}", tag="x8")
                    x8a = x8[:, :, :, :]
                    src8 = bass.AP(
                        x8a.tensor, x8a.offset + t0,
                        [[4 * TOK_PAD8, 128], [TOK_PAD8, 4], [1, OCT_W8]])
                    nc.sync.dma_start(out=x8t, in_=src8)
                xtile = x_pool.tile([128, 4, OCT_W], BF, name=f"xt{o}",
                                    tag="xt")
                xTa = xT[:, :]
                srcx = bass.AP(
                    xTa.tensor, xTa.offset + t0,
                    [[TOK_PAD, 128], [128 * TOK_PAD, 4], [1, OCT_W]])
                nc.sync.dma_start(out=xtile, in_=srcx)
                xts = [xtile[:, ci, :] for ci in range(4)]
                # slot-expanded copies for the V stationary (weights APs may
                # only have one free dim); SBUF->SBUF so Pool can do them
                xvs = []
                for ci in range(4):
                    xv = x_pool.tile([128, 4, 128], BF, name=f"xv{o}_{ci}",
                                     tag=f"xv{ci}")
                    xr = xts[ci]
                    src = bass.AP(xr.tensor, xr.offset,
                                  [xr.ap[0], [98, 4], [P, 2], [1, 64]])
                    nc.gpsimd.tensor_copy(xv, src)
                    xvs.append(xv)

                # QK projection: q tiles feature-major [128, OCT_W]; k tiles
                # slot-expanded [128, 4pair, 128] for contiguous stationaries.
                # ft order interleaves q/k so score matmuls unblock early.
                qs = [None] * 4
                ks = [None] * 4
                for ft in (0, 4, 1, 5, 2, 6, 3, 7):
                    ps = ps_head.tile([128, OCT_W], F32, name=f"qkp{o}_{ft}",
                                      tag="head")
                    if USE_FP8:
                        for ci in range(2):
                            nc.tensor.matmul(
                                ps, w8_sb[ci][:, :, 128 * ft:128 * (ft + 1)],
                                x8t[:, ci, :, 0:OCT_W],
                                start=(ci == 0),
                                stop=(ci == 1 and not has_bqk),
                                perf_mode=DR)
                    else:
                        for ci in range(4):
                            nc.tensor.matmul(
                                ps, wqk_sb[ci][:, 128 * ft:128 * (ft + 1)],
                                xts[ci], start=(ci == 0),
                                stop=(ci == 3 and not has_bqk))
                    if has_bqk:
                        nc.tensor.matmul(ps, bqk_sb[:, 128 * ft:128 * (ft + 1)],
                                         ones_row, start=False, stop=True)
                    if ft < 4:
                        q_sb = qk_pool.tile([128, OCT_TOK], BF, name=f"q{o}_{ft}",
                                            tag=f"q{ft}")
                        nc.scalar.copy(q_sb, ps[:, 0:OCT_TOK])
                        qs[ft] = q_sb
                    else:
                        k_sb = qk_pool.tile([128, 4, 128], BF, name=f"k{o}_{ft}",
                                            tag=f"k{ft}")
                        src = bass.AP(ps.tensor, ps.offset,
                                      [ps.ap[0], [98, 4], [P, 2], [1, 64]])
                        nc.vector.tensor_copy(k_sb, src)
                        ks[ft - 4] = k_sb

                # V projection (all pairs; slot-expanded stationary)
                v_sbs = []
                for p in range(4):
                    vps = ps_head.tile([128, C], F32, name=f"vp{o}_{p}",
                                       tag="head")
                    for ci in range(4):
                        nc.tensor.matmul(
                            vps, xvs[ci][:, p, :],
                            wv_sb[ci], start=(ci == 0),
                            stop=(ci == 3 and not has_bv))
                    if has_bv:
                        nc.tensor.matmul(vps, ones_row[:, 0:128], bv_sb,
                                         start=False, stop=True)
                    v_sb = v_pool.tile([128, NH, 33], BF, name=f"v{o}_{p}",
                                       tag="v")
                    vv = vps.rearrange("q (h d) -> q h d", h=NH)
                    nc.scalar.copy(v_sb[:, :, 0:32], vv)
                    v_sbs.append(v_sb)

                # scores: one matmul per (pair, head) covering both windows,
                # all 16 into one 4-bank PSUM tile (j selects the bank, so
                # each tile_position row group owns its own bank); row groups
                # rotate (j innermost) so LDWEIGHTS overlaps
                for p in range(4):
                    pt0 = 98 * p
                    stp = ps_st.tile([128, 4, 4, 128], F32, name=f"st{o}_{p}",
                                     tag="st")
                    sesp = se_pool.tile([128, 4, 4, 128], BF,
                                        name=f"se{o}_{p}", tag="se")
                    for i in range(4):
                        for j in range(4):
                            r = 32 * j
                            nc.tensor.matmul(
                                stp[:, j, i, 0:98],
                                ks[i][r:r + 32, p, :],
                                qs[i][r:r + 32, pt0:pt0 + 98],
                                start=True, stop=True,
                                tile_position=(r, 0))
                    with tc.high_priority():
                        nc.scalar.activation(
                            out=sesp[0:P, :, :, 0:P], in_=stp[0:P, :, :, 0:P],
                            func=Exp, scale=ESC)
                        nc.scalar.activation(
                            out=sesp[64:64 + P, :, :, 64:64 + P],
                            in_=stp[64:64 + P, :, :, P:2 * P],
                            func=Exp, scale=ESC)
                    # multiplicative rel-pos bias on the two diagonal
                    # blocks; j-halves so AV g=0 (j 0,1) releases early
                    for jh in range(2):
                        js = slice(2 * jh, 2 * jh + 2)
                        nc.gpsimd.tensor_mul(sesp[0:P, js, :, 0:P],
                                             sesp[0:P, js, :, 0:P],
                                             eb_sb[0:P, js, :, :])
                        nc.vector.tensor_mul(sesp[64:64 + P, js, :, 64:64 + P],
                                             sesp[64:64 + P, js, :, 64:64 + P],
                                             eb_sb[64:64 + P, js, :, :])

                    # AV: one matmul per head over the full 128-partition
                    # block-diagonal stationary; 33rd v column accumulates
                    # the softmax denominator
                    v_sb = v_sbs[p]
                    attn_sb = attn_pool.tile([128, NH, HD], BF,
                                             name=f"attn{o}_{p}", tag="attn")
                    for g in range(2):
                        # heads h = 4i + 2g + jj; one avt bank cycles
                        # av(g=0) -> av(g=1) -> atp within each pair
                        av = ps_avt.tile([128, 4, 2, 33], F32,
                                         name=f"av{o}_{p}_{g}", tag="avt")
                        for i in range(4):
                            for jj in range(2):
                                h = 4 * i + 2 * g + jj
                                nc.tensor.matmul(av[:, i, jj, :],
                                                 sesp[:, 2 * g + jj, i, :],
                                                 v_sb[:, h, :],
                                                 start=True, stop=True)
                        zr = zr_pool.tile([128, 4, 2], F32,
                                          name=f"zr{o}_{p}_{g}", tag="zr")
                        with tc.high_priority():
                            nc.vector.reciprocal(zr, av[:, :, :, 32])
                            attn_v = bass.AP(attn_sb.tensor,
                                             attn_sb.offset + 64 * g,
                                             [attn_sb.ap[0], [128, 4], [32, 2],
                                              [1, HD]])
                            nc.vector.tensor_mul(attn_v, av[:, :, :, 0:32],
                                                 bcast_last(zr, HD))

                    # transpose to feature-major (po bank; av recycles sooner)
                    atp = ps_po.tile([128, 4, 128], BF, name=f"atp{o}_{p}",
                                     tag="po")
                    attn_flat = attn_sb.rearrange("q h d -> q (h d)")
                    for ci in range(4):
                        nc.tensor.transpose(atp[:, ci, :],
                                            attn_flat[:, 128 * ci:128 * (ci + 1)],
                                            ident)
                    if p == 0:
                        atT = atT_pool.tile([128, 4, 4, 128], BF,
                                            name=f"atT{o}", tag="atT")
                    dst = bass.AP(atT.tensor, atT.offset + 128 * p,
                                  [atT.ap[0], [512, 4], [1, 128]])
                    with tc.high_priority():
                        nc.vector.tensor_copy(dst, atp)

                # output projection, feature-major; the moving AP gathers the
                # 49-token windows out of the 64-token slots (dense 392 cols)
                osb = out_pool.tile([128, 4, OCT_TOK], BF, name=f"o{o}",
                                    tag="osb")
                for et in range(4):
                    poT = ps_po.tile([128, OCT_TOK], F32, name=f"po{o}_{et}",
                                     tag="po")
                    for ci in range(4):
                        rhs = bass.AP(atT.tensor, atT.offset + 512 * ci,
                                      [atT.ap[0], [128, 4], [64, 2], [1, P]])
                        nc.tensor.matmul(poT, wp_sb[ci][:, 128 * et:128 * (et + 1)],
                                         rhs, start=(ci == 0),
                                         stop=(ci == 3 and not has_bp))
                    if has_bp:
                        nc.tensor.matmul(poT, bpT_sb[:, 128 * et:128 * (et + 1)],
                                         ones_row[:, 0:OCT_TOK],
                                         start=False, stop=True)
                    if et < 2:
                        nc.scalar.copy(osb[:, et, :], poT)
                    else:
                        nc.vector.tensor_copy(osb[:, et, :], poT)
                # single store per octet on the Activation DGE ring
                outa = out[:, :]
                dst = bass.AP(outa.tensor, outa.offset + t0,
                              [[TOK, 128], [128 * TOK, 4], [1, OCT_TOK]])
                nc.scalar.dma_start(out=dst, in_=osb)
    return nc


def _host_prep(x, qkv_w, qkv_b, proj_w, proj_b, rpb_table, rel_index):
    scale = HD ** -0.5
    # weights: qkv feature order is (3, NH, HD) -> q=0:512, k=512:1024, v=1024:1536
    wq = qkv_w[0:C, :] * scale          # fold attention scale into q
    wk = qkv_w[C:2 * C, :]
    wv = qkv_w[2 * C:3 * C, :]
    wv_t = np.ascontiguousarray(wv.T).astype(BF16)              # [C, C]
    wp_t = np.ascontiguousarray(proj_w.T).astype(BF16)          # [C, C]

    bq = qkv_b[0:C] * scale
    bk = qkv_b[C:2 * C]
    if USE_FP8:
        wqk = np.concatenate([wq.T * SQ, wk.T * SK], axis=1)    # [C, 2C]
        # feature f = ci*256 + i*128 + p  ->  w8[ci][p, i, m]
        w8 = np.ascontiguousarray(
            wqk.reshape(2, 2, 128, 2 * C).transpose(0, 2, 1, 3)).astype(FP8)
        bqk = np.concatenate([bq * SQ, bk * SK])[None, :].astype(BF16)
    else:
        wqk = np.concatenate([wq.T, wk.T], axis=1).astype(BF16)
        bqk = np.concatenate([bq, bk])[None, :].astype(BF16)

    bias = rpb_table[rel_index]                  # [n, m, NH]
    biasT = np.transpose(bias, (2, 1, 0))        # [h, m, n]
    ebias = np.exp(biasT.astype(np.float64)).astype(np.float32)
    ebp = np.ones((128, NH, P), np.float32)
    ebp[0:P] = np.transpose(ebias, (1, 0, 2))    # rows 0:49  (window slot 0)
    ebp[64:64 + P] = ebp[0:P]                    # rows 64:113 (window slot 1)
    # regroup heads: [128, j, i, P] with head 4*i + j at [:, j, i, :]
    eb = np.ascontiguousarray(
        ebp.reshape(128, 4, 4, P).transpose(0, 2, 1, 3)).astype(BF16)

    bv_ = qkv_b[2 * C:3 * C]
    bv = bv_[None, :].astype(BF16)
    bp = proj_b[None, :].astype(BF16)

    has_bqk = bool(np.any(qkv_b[0:2 * C] != 0))
    has_bv = bool(np.any(bv_ != 0))
    has_bp = bool(np.any(proj_b != 0))

    in_maps = []
    for c in range(NCORES):
        xc = np.asarray(x[c * B_CORE:(c + 1) * B_CORE]).reshape(TOK, C)
        xTc = np.zeros((C, TOK_PAD), np.float32)
        xTc[:, :TOK] = xc.T
        m = {"xT": xTc.astype(BF16), "wv": wv_t, "wp": wp_t, "eb": eb}
        if USE_FP8:
            x8c = np.zeros((C, TOK_PAD8), np.float32)
            x8c[:, :TOK] = xc.T
            m["x8"] = np.ascontiguousarray(
                x8c.reshape(2, 2, 128, TOK_PAD8).transpose(2, 0, 1, 3)).astype(FP8)
            m["w8"] = w8
        else:
            m["wqk"] = wqk
        if has_bqk:
            m["bqk"] = bqk
        if has_bv:
            m["bv"] = bv
        if has_bp:
            m["bp"] = bp
        in_maps.append(m)
    return in_maps, has_bqk, has_bv, has_bp


def kernel(x, qkv_w, qkv_b, proj_w, proj_b, rpb_table, rel_index):
    from concourse import bacc
    from concourse.bass_utils import run_bass_kernel_spmd

    in_maps, has_bqk, has_bv, has_bp = _host_prep(
        np.asarray(x, np.float32), np.asarray(qkv_w, np.float32),
        np.asarray(qkv_b, np.float32), np.asarray(proj_w, np.float32),
        np.asarray(proj_b, np.float32), np.asarray(rpb_table, np.float32),
        np.asarray(rel_index))

    nc = bacc.Bacc()
    _build(nc, has_bqk, has_bv, has_bp)
    nc.finalize()

    trace = os.environ.get("BASS_KERNEL_TRACE", "") == "1"
    res = run_bass_kernel_spmd(nc, in_maps, core_ids=list(range(NCORES)),
                               trace=trace)
    if trace and res.exec_time_ns is not None:
        print(f"HW exec time: {res.exec_time_ns} ns", flush=True)

    outs = [r["out"].astype(np.float32).T.reshape(B_CORE, P, C)
            for r in res.results]
    return np.concatenate(outs, axis=0)


# revision 5
# speedup vs baseline: 1.1862x; 1.0043x over previous
"""Swin windowed attention (B_=2048 windows, N=49 tokens, C=512, 16 heads)
on 8 Trainium2 NeuronCores, data-parallel over windows (256 windows/core).

Per-core layout (v3; ~1.4x fewer PE instructions and ~35%% less engine work
than the v1 baseline, with input/output DMA rings decoupled):

  - q,k projection runs in fp8e4 with DoubleRow perf mode (K=256/pass, 2
    passes): 16 matmuls/octet instead of 32.  Host folds the attention
    scale and power-of-2 fp8 scales (q x256, k x64) into the weights; the
    combined descale rides the exp activation's scale operand for free.
  - scores^T are computed two-windows-per-matmul: the stationary is a
    [32, 128] slice of a "kslot" tile whose columns are the pair's two
    windows at 64-aligned slots (materialized contiguously so FWL stays
    on); the moving q covers both windows (98 cols).  64 score matmuls
    per octet at 4 rotating row groups (LDWEIGHTS overlaps), all 16 of a
    pair accumulating into one 4-bank PSUM tile (bank = tile_position row).
  - exp'd scores land in zero-padded block-diagonal "sesp" tiles
    [128, 4j, 4i, 128] (w0 at [0:49, ..., 0:49], w1 at [64:113, ..., 64:113],
    zeros memset once per pool buffer).  One AV matmul per (pair, head)
    contracts all 128 partitions with the contiguous [128, 128] stationary:
    64 AV matmuls/octet; a 33rd ones-column of v accumulates the softmax
    denominator.
  - v projection uses slot-expanded x copies produced on the otherwise-idle
    GPSIMD engine (SBUF->SBUF; PSUM is GPSIMD-inaccessible, and matmul
    weights APs only allow one free dim).
  - normalization = DVE reciprocal + free-broadcast multiply; attention
    output is PE-transposed to feature-major, and the output projection is
    emitted feature-major with the moving AP gathering 49-token windows out
    of the 64-token slots; out^T [C, TOK] is un-transposed on the host.
  - engine budget: ACT exp/q/v/osb, DVE kslot/eb-w1/recip/attn/atT,
    GPSIMD xv/eb-w0/memsets.  PSUM: 2 banks QK+V, 4 banks scores, 1 bank
    AV+transpose, 1 bank projection - the head of octet o+1 shares nothing
    with the tail of octet o.  Inputs ride the SP DGE ring (one fused DMA
    per tensor per octet), the single output DMA rides the ACT ring.
"""

import os
import sys

import numpy as np
import ml_dtypes

if "/opt/trn_rl_repo" not in sys.path:
    sys.path.insert(0, "/opt/trn_rl_repo")

P = 49          # tokens per window
NH = 16         # heads
HD = 32         # head dim
C = 512         # model dim
NCORES = 8
B_TOTAL = 2048
B_CORE = B_TOTAL // NCORES        # 256 windows per core
TOK = B_CORE * P                  # 12544 tokens per core
OCT = 32                          # octets (8 windows) per core
OCT_TOK = 8 * P                   # 392 tokens per octet
OCT_W = OCT_TOK + 16              # octet tile width incl 16-token overlap
OCT_W8 = OCT_TOK + 24             # fp8 tile width (dim-1 step must be %16)
TOK_PAD = TOK + 16
TOK_PAD8 = TOK + 32
BF16 = ml_dtypes.bfloat16
FP8 = ml_dtypes.float8_e4m3fn

USE_FP8 = True
SQ = 256.0     # fp8 scale folded into q weights (power of 2)
SK = 64.0      # fp8 scale folded into k weights
ESC = 1.0 / (SQ * SK) if USE_FP8 else 1.0


def _build(nc, has_bqk, has_bv, has_bp, n_oct=OCT):
    import concourse.bass as bass
    import concourse.mybir as mybir
    from concourse.tile import TileContext
    from concourse.masks import make_identity

    F32 = mybir.dt.float32
    BF = mybir.dt.bfloat16
    F8 = mybir.dt.float8e4
    Exp = mybir.ActivationFunctionType.Exp
    DR = mybir.MatmulPerfMode.DoubleRow

    if USE_FP8:
        x8 = nc.dram_tensor("x8", [128, 2, 2, TOK_PAD8], F8, kind="ExternalInput")
        w8 = nc.dram_tensor("w8", [2, 128, 2, 2 * C], F8, kind="ExternalInput")
    else:
        wqk = nc.dram_tensor("wqk", [C, 2 * C], BF, kind="ExternalInput")
    xT = nc.dram_tensor("xT", [C, TOK_PAD], BF, kind="ExternalInput")
    wv = nc.dram_tensor("wv", [C, C], BF, kind="ExternalInput")
    wp = nc.dram_tensor("wp", [C, C], BF, kind="ExternalInput")
    eb = nc.dram_tensor("eb", [128, 4, 4, P], BF, kind="ExternalInput")
    bqk = bv = bp = None
    if has_bqk:
        bqk = nc.dram_tensor("bqk", [1, 2 * C], BF, kind="ExternalInput")
    if has_bv:
        bv = nc.dram_tensor("bv", [1, C], BF, kind="ExternalInput")
    if has_bp:
        bp = nc.dram_tensor("bp", [1, C], BF, kind="ExternalInput")
    out = nc.dram_tensor("out", [C, TOK], BF, kind="ExternalOutput")

    def bcast_last(ap, n):
        return bass.AP(ap.tensor, ap.offset, [*ap.ap, [0, n]])

    with TileContext(nc) as tc:
        with (
            tc.tile_pool(name="singles", bufs=1) as singles,
            tc.tile_pool(name="xin", bufs=2) as x_pool,
            tc.tile_pool(name="qk", bufs=2) as qk_pool,
            tc.tile_pool(name="vsb", bufs=6) as v_pool,
            tc.tile_pool(name="se", bufs=3) as se_pool,
            tc.tile_pool(name="zr", bufs=8) as zr_pool,
            tc.tile_pool(name="attn", bufs=6) as attn_pool,
            tc.tile_pool(name="att", bufs=3) as atT_pool,
            tc.tile_pool(name="osb", bufs=4) as out_pool,
            tc.tile_pool(name="ps_head", bufs=2, space="PSUM") as ps_head,
            tc.tile_pool(name="ps_st", bufs=1, space="PSUM") as ps_st,
            tc.tile_pool(name="ps_avt", bufs=1, space="PSUM") as ps_avt,
            tc.tile_pool(name="ps_po", bufs=1, space="PSUM") as ps_po,
        ):
            # --- constants / weights ---
            if USE_FP8:
                w8_sb = []
                for ci in range(2):
                    w8_t = singles.tile([128, 2, 2 * C], F8, name=f"w8{ci}")
                    nc.sync.dma_start(out=w8_t, in_=w8[ci, :, :, :])
                    w8_sb.append(w8_t)
            else:
                wqk_sb = []
                for ci in range(4):
                    wqk_t = singles.tile([128, 2 * C], BF, name=f"wqk{ci}")
                    nc.sync.dma_start(out=wqk_t, in_=wqk[128 * ci:128 * (ci + 1), :])
                    wqk_sb.append(wqk_t)
            wv_sb = []
            wp_sb = []
            for ci in range(4):
                wv_t = singles.tile([128, C], BF, name=f"wv{ci}")
                nc.sync.dma_start(out=wv_t, in_=wv[128 * ci:128 * (ci + 1), :])
                wv_sb.append(wv_t)
                wp_t = singles.tile([128, C], BF, name=f"wp{ci}")
                nc.sync.dma_start(out=wp_t, in_=wp[128 * ci:128 * (ci + 1), :])
                wp_sb.append(wp_t)
            eb_sb = singles.tile([128, 4, 4, P], BF, name="ebsb")
            nc.sync.dma_start(out=eb_sb, in_=eb[:, :, :, :])
            ident = singles.tile([128, 128], BF, name="ident")
            make_identity(nc, ident)
            bqk_sb = bv_sb = bpT_sb = ones_row = None
            if has_bqk or has_bv or has_bp:
                ones_row = singles.tile([1, OCT_W], BF, name="onesrow")
                nc.vector.memset(ones_row, 1.0)
            if has_bqk:
                bqk_sb = singles.tile([1, 2 * C], BF, name="bqksb")
                nc.sync.dma_start(out=bqk_sb, in_=bqk[:, :])
            if has_bv:
                bv_sb = singles.tile([1, C], BF, name="bvsb")
                nc.sync.dma_start(out=bv_sb, in_=bv[:, :])
            if has_bp:
                bpT_sb = singles.tile([1, C], BF, name="bpsb")
                nc.sync.dma_start(out=bpT_sb, in_=bp[:, :])

            # pre-zero the block-diagonal ses tiles (exp/eb rewrite only the
            # two diagonal blocks; the zero padding is what makes the fused
            # 2-window AV stationary sound) and pre-set the v denominator
            # ones column (the v copy writes only [:, :, 0:32]).
            for b in range(3):
                sez = se_pool.tile([128, 4, 4, 128], BF, name=f"sez{b}",
                                   tag="se")
                nc.gpsimd.memset(sez, 0.0)
            for b in range(6):
                vz = v_pool.tile([128, NH, 33], BF, name=f"vz{b}", tag="v")
                nc.gpsimd.memset(vz, 1.0)

            # --- main loop ---
            for o in range(n_oct):
                t0 = o * OCT_TOK
                # input DMAs (SP ring only carries inputs; outputs go via
                # the Activation ring so next-octet loads are never queued
                # behind this octet's stores)
                if USE_FP8:
                    x8t = x_pool.tile([128, 2, 2, OCT_W8], F8,
                                      name=f"x8{o}", tag="x8")
                    x8a = x8[:, :, :, :]
                    src8 = bass.AP(
                        x8a.tensor, x8a.offset + t0,
                        [[4 * TOK_PAD8, 128], [TOK_PAD8, 4], [1, OCT_W8]])
                    nc.sync.dma_start(out=x8t, in_=src8)
                xtile = x_pool.tile([128, 4, OCT_W], BF, name=f"xt{o}",
                                    tag="xt")
                xTa = xT[:, :]
                srcx = bass.AP(
                    xTa.tensor, xTa.offset + t0,
                    [[TOK_PAD, 128], [128 * TOK_PAD, 4], [1, OCT_W]])
                nc.sync.dma_start(out=xtile, in_=srcx)
                xts = [xtile[:, ci, :] for ci in range(4)]
                # slot-expanded copies for the V stationary (weights APs may
                # only have one free dim); SBUF->SBUF so Pool can do them
                xvs = []
                for ci in range(4):
                    xv = x_pool.tile([128, 4, 128], BF, name=f"xv{o}_{ci}",
                                     tag=f"xv{ci}")
                    xr = xts[ci]
                    src = bass.AP(xr.tensor, xr.offset,
                                  [xr.ap[0], [98, 4], [P, 2], [1, 64]])
                    nc.gpsimd.tensor_copy(xv, src)
                    xvs.append(xv)

                # QK projection: q tiles feature-major [128, OCT_W]; k tiles
                # slot-expanded [128, 4pair, 128] for contiguous stationaries.
                # ft order interleaves q/k so score matmuls unblock early.
                qs = [None] * 4
                ks = [None] * 4
                for ft in (0, 4, 1, 5, 2, 6, 3, 7):
                    ps = ps_head.tile([128, OCT_W], F32, name=f"qkp{o}_{ft}",
                                      tag="head")
                    if USE_FP8:
                        for ci in range(2):
                            nc.tensor.matmul(
                                ps, w8_sb[ci][:, :, 128 * ft:128 * (ft + 1)],
                                x8t[:, ci, :, 0:OCT_W],
                                start=(ci == 0),
                                stop=(ci == 1 and not has_bqk),
                                perf_mode=DR)
                    else:
                        for ci in range(4):
                            nc.tensor.matmul(
                                ps, wqk_sb[ci][:, 128 * ft:128 * (ft + 1)],
                                xts[ci], start=(ci == 0),
                                stop=(ci == 3 and not has_bqk))
                    if has_bqk:
                        nc.tensor.matmul(ps, bqk_sb[:, 128 * ft:128 * (ft + 1)],
                                         ones_row, start=False, stop=True)
                    if ft < 4:
                        q_sb = qk_pool.tile([128, OCT_TOK], BF, name=f"q{o}_{ft}",
                                            tag=f"q{ft}")
                        nc.scalar.copy(q_sb, ps[:, 0:OCT_TOK])
                        qs[ft] = q_sb
                    else:
                        k_sb = qk_pool.tile([128, 4, 128], BF, name=f"k{o}_{ft}",
                                            tag=f"k{ft}")
                        src = bass.AP(ps.tensor, ps.offset,
                                      [ps.ap[0], [98, 4], [P, 2], [1, 64]])
                        nc.vector.tensor_copy(k_sb, src)
                        ks[ft - 4] = k_sb

                # V projection (all pairs; slot-expanded stationary)
                v_sbs = []
                for p in range(4):
                    vps = ps_head.tile([128, C], F32, name=f"vp{o}_{p}",
                                       tag="head")
                    for ci in range(4):
                        nc.tensor.matmul(
                            vps, xvs[ci][:, p, :],
                            wv_sb[ci], start=(ci == 0),
                            stop=(ci == 3 and not has_bv))
                    if has_bv:
                        nc.tensor.matmul(vps, ones_row[:, 0:128], bv_sb,
                                         start=False, stop=True)
                    v_sb = v_pool.tile([128, NH, 33], BF, name=f"v{o}_{p}",
                                       tag="v")
                    vv = vps.rearrange("q (h d) -> q h d", h=NH)
                    nc.scalar.copy(v_sb[:, :, 0:32], vv)
                    v_sbs.append(v_sb)

                # scores: one matmul per (pair, head) covering both windows,
                # all 16 into one 4-bank PSUM tile (j selects the bank, so
                # each tile_position row group owns its own bank); row groups
                # rotate (j innermost) so LDWEIGHTS overlaps
                for p in range(4):
                    pt0 = 98 * p
                    stp = ps_st.tile([128, 4, 4, 128], F32, name=f"st{o}_{p}",
                                     tag="st")
                    sesp = se_pool.tile([128, 4, 4, 128], BF,
                                        name=f"se{o}_{p}", tag="se")
                    for i in range(4):
                        for j in range(4):
                            r = 32 * j
                            nc.tensor.matmul(
                                stp[:, j, i, 0:98],
                                ks[i][r:r + 32, p, :],
                                qs[i][r:r + 32, pt0:pt0 + 98],
                                start=True, stop=True,
                                tile_position=(r, 0))
                    with tc.high_priority():
                        nc.scalar.activation(
                            out=sesp[0:P, :, :, 0:P], in_=stp[0:P, :, :, 0:P],
                            func=Exp, scale=ESC)
                        nc.scalar.activation(
                            out=sesp[64:64 + P, :, :, 64:64 + P],
                            in_=stp[64:64 + P, :, :, P:2 * P],
                            func=Exp, scale=ESC)
                    # multiplicative rel-pos bias on the two diagonal
                    # blocks; j-halves so AV g=0 (j 0,1) releases early
                    for jh in range(2):
                        js = slice(2 * jh, 2 * jh + 2)
                        nc.gpsimd.tensor_mul(sesp[0:P, js, :, 0:P],
                                             sesp[0:P, js, :, 0:P],
                                             eb_sb[0:P, js, :, :])
                        nc.vector.tensor_mul(sesp[64:64 + P, js, :, 64:64 + P],
                                             sesp[64:64 + P, js, :, 64:64 + P],
                                             eb_sb[64:64 + P, js, :, :])

                    # AV: one matmul per head over the full 128-partition
                    # block-diagonal stationary; 33rd v column accumulates
                    # the softmax denominator
                    v_sb = v_sbs[p]
                    attn_sb = attn_pool.tile([128, NH, HD], BF,
                                             name=f"attn{o}_{p}", tag="attn")
                    for g in range(2):
                        # heads h = 4i + 2g + jj; one avt bank cycles
                        # av(g=0) -> av(g=1) -> atp within each pair
                        av = ps_avt.tile([128, 4, 2, 33], F32,
                                         name=f"av{o}_{p}_{g}", tag="avt")
                        for i in range(4):
                            for jj in range(2):
                                h = 4 * i + 2 * g + jj
                                nc.tensor.matmul(av[:, i, jj, :],
                                                 sesp[:, 2 * g + jj, i, :],
                                                 v_sb[:, h, :],
                                                 start=True, stop=True)
                        zr = zr_pool.tile([128, 4, 2], F32,
                                          name=f"zr{o}_{p}_{g}", tag="zr")
                        with tc.high_priority():
                            nc.vector.reciprocal(zr, av[:, :, :, 32])
                            attn_v = bass.AP(attn_sb.tensor,
                                             attn_sb.offset + 64 * g,
                                             [attn_sb.ap[0], [128, 4], [32, 2],
                                              [1, HD]])
                            nc.vector.tensor_mul(attn_v, av[:, :, :, 0:32],
                                                 bcast_last(zr, HD))

                    # transpose to feature-major (po bank; av recycles sooner)
                    atp = ps_po.tile([128, 4, 128], BF, name=f"atp{o}_{p}",
                                     tag="po")
                    attn_flat = attn_sb.rearrange("q h d -> q (h d)")
                    for ci in range(4):
                        nc.tensor.transpose(atp[:, ci, :],
                                            attn_flat[:, 128 * ci:128 * (ci + 1)],
                                            ident)
                    if p == 0:
                        atT = atT_pool.tile([128, 4, 4, 128], BF,
                                            name=f"atT{o}", tag="atT")
                    dst = bass.AP(atT.tensor, atT.offset + 128 * p,
                                  [atT.ap[0], [512, 4], [1, 128]])
                    with tc.high_priority():
                        nc.vector.tensor_copy(dst, atp)

                # output projection, feature-major; the moving AP gathers the
                # 49-token windows out of the 64-token slots (dense 392 cols)
                osb = out_pool.tile([128, 4, OCT_TOK], BF, name=f"o{o}",
                                    tag="osb")
                for et in range(4):
                    poT = ps_po.tile([128, OCT_TOK], F32, name=f"po{o}_{et}",
                                     tag="po")
                    for ci in range(4):
                        rhs = bass.AP(atT.tensor, atT.offset + 512 * ci,
                                      [atT.ap[0], [128, 4], [64, 2], [1, P]])
                        nc.tensor.matmul(poT, wp_sb[ci][:, 128 * et:128 * (et + 1)],
                                         rhs, start=(ci == 0),
                                         stop=(ci == 3 and not has_bp))
                    if has_bp:
                        nc.tensor.matmul(poT, bpT_sb[:, 128 * et:128 * (et + 1)],
                                         ones_row[:, 0:OCT_TOK],
                                         start=False, stop=True)
                    if et < 2:
                        nc.scalar.copy(osb[:, et, :], poT)
                    else:
                        nc.vector.tensor_copy(osb[:, et, :], poT)
                # single store per octet on the Activation DGE ring
                outa = out[:, :]
                dst = bass.AP(outa.tensor, outa.offset + t0,
                              [[TOK, 128], [128 * TOK, 4], [1, OCT_TOK]])
                nc.scalar.dma_start(out=dst, in_=osb)
    return nc


def _host_prep(x, qkv_w, qkv_b, proj_w, proj_b, rpb_table, rel_index):
    scale = HD ** -0.5
    # weights: qkv feature order is (3, NH, HD) -> q=0:512, k=512:1024, v=1024:1536
    wq = qkv_w[0:C, :] * scale          # fold attention scale into q
    wk = qkv_w[C:2 * C, :]
    wv = qkv_w[2 * C:3 * C, :]
    wv_t = np.ascontiguousarray(wv.T).astype(BF16)              # [C, C]
    wp_t = np.ascontiguousarray(proj_w.T).astype(BF16)          # [C, C]

    bq = qkv_b[0:C] * scale
    bk = qkv_b[C:2 * C]
    if USE_FP8:
        wqk = np.concatenate([wq.T * SQ, wk.T * SK], axis=1)    # [C, 2C]
        # feature f = ci*256 + i*128 + p  ->  w8[ci][p, i, m]
        w8 = np.ascontiguousarray(
            wqk.reshape(2, 2, 128, 2 * C).transpose(0, 2, 1, 3)).astype(FP8)
        bqk = np.concatenate([bq * SQ, bk * SK])[None, :].astype(BF16)
    else:
        wqk = np.concatenate([wq.T, wk.T], axis=1).astype(BF16)
        bqk = np.concatenate([bq, bk])[None, :].astype(BF16)

    bias = rpb_table[rel_index]                  # [n, m, NH]
    biasT = np.transpose(bias, (2, 1, 0))        # [h, m, n]
    ebias = np.exp(biasT.astype(np.float64)).astype(np.float32)
    ebp = np.ones((128, NH, P), np.float32)
    ebp[0:P] = np.transpose(ebias, (1, 0, 2))    # rows 0:49  (window slot 0)
    ebp[64:64 + P] = ebp[0:P]                    # rows 64:113 (window slot 1)
    # regroup heads: [128, j, i, P] with head 4*i + j at [:, j, i, :]
    eb = np.ascontiguousarray(
        ebp.reshape(128, 4, 4, P).transpose(0, 2, 1, 3)).astype(BF16)

    bv_ = qkv_b[2 * C:3 * C]
    bv = bv_[None, :].astype(BF16)
    bp = proj_b[None, :].astype(BF16)

    has_bqk = bool(np.any(qkv_b[0:2 * C] != 0))
    has_bv = bool(np.any(bv_ != 0))
    has_bp = bool(np.any(proj_b != 0))

    in_maps = []
    for c in range(NCORES):
        xc = np.asarray(x[c * B_CORE:(c + 1) * B_CORE]).reshape(TOK, C)
        xTc = np.zeros((C, TOK_PAD), np.float32)
        xTc[:, :TOK] = xc.T
        m = {"xT": xTc.astype(BF16), "wv": wv_t, "wp": wp_t, "eb": eb}
        if USE_FP8:
            x8c = np.zeros((C, TOK_PAD8), np.float32)
            x8c[:, :TOK] = xc.T
            m["x8"] = np.ascontiguousarray(
                x8c.reshape(2, 2, 128, TOK_PAD8).transpose(2, 0, 1, 3)).astype(FP8)
            m["w8"] = w8
        else:
            m["wqk"] = wqk
        if has_bqk:
            m["bqk"] = bqk
        if has_bv:
            m["bv"] = bv
        if has_bp:
            m["bp"] = bp
        in_maps.append(m)
    return in_maps, has_bqk, has_bv, has_bp


def kernel(x, qkv_w, qkv_b, proj_w, proj_b, rpb_table, rel_index):
    from concourse import bacc
    from concourse.bass_utils import run_bass_kernel_spmd

    in_maps, has_bqk, has_bv, has_bp = _host_prep(
        np.asarray(x, np.float32), np.asarray(qkv_w, np.float32),
        np.asarray(qkv_b, np.float32), np.asarray(proj_w, np.float32),
        np.asarray(proj_b, np.float32), np.asarray(rpb_table, np.float32),
        np.asarray(rel_index))

    nc = bacc.Bacc()
    _build(nc, has_bqk, has_bv, has_bp)
    nc.finalize()

    trace = os.environ.get("BASS_KERNEL_TRACE", "") == "1"
    res = run_bass_kernel_spmd(nc, in_maps, core_ids=list(range(NCORES)),
                               trace=trace)
    if trace and res.exec_time_ns is not None:
        print(f"HW exec time: {res.exec_time_ns} ns", flush=True)

    outs = [r["out"].astype(np.float32).T.reshape(B_CORE, P, C)
            for r in res.results]
    return np.concatenate(outs, axis=0)


# revision 6
# speedup vs baseline: 1.2741x; 1.0741x over previous
"""Swin windowed attention (B_=2048 windows, N=49 tokens, C=512, 16 heads)
on 8 Trainium2 NeuronCores, data-parallel over windows (256 windows/core).

Per-core layout (v3; ~1.4x fewer PE instructions and ~35%% less engine work
than the v1 baseline, with input/output DMA rings decoupled):

  - q,k projection runs in fp8e4 with DoubleRow perf mode (K=256/pass, 2
    passes): 16 matmuls/octet instead of 32.  Host folds the attention
    scale and power-of-2 fp8 scales (q x256, k x64) into the weights; the
    combined descale rides the exp activation's scale operand for free.
  - scores^T are computed two-windows-per-matmul: the stationary is a
    [32, 128] slice of a "kslot" tile whose columns are the pair's two
    windows at 64-aligned slots (materialized contiguously so FWL stays
    on); the moving q covers both windows (98 cols).  64 score matmuls
    per octet at 4 rotating row groups (LDWEIGHTS overlaps), all 16 of a
    pair accumulating into one 4-bank PSUM tile (bank = tile_position row).
  - exp'd scores land in zero-padded block-diagonal "sesp" tiles
    [128, 4j, 4i, 128] (w0 at [0:49, ..., 0:49], w1 at [64:113, ..., 64:113],
    zeros memset once per pool buffer).  One AV matmul per (pair, head)
    contracts all 128 partitions with the contiguous [128, 128] stationary:
    64 AV matmuls/octet; a 33rd ones-column of v accumulates the softmax
    denominator.
  - v projection uses slot-expanded x copies produced on the otherwise-idle
    GPSIMD engine (SBUF->SBUF; PSUM is GPSIMD-inaccessible, and matmul
    weights APs only allow one free dim).
  - normalization = DVE reciprocal + free-broadcast multiply; attention
    output is PE-transposed to feature-major, and the output projection is
    emitted feature-major with the moving AP gathering 49-token windows out
    of the 64-token slots; out^T [C, TOK] is un-transposed on the host.
  - engine budget: ACT exp/q/v/osb, DVE kslot/eb-w1/recip/attn/atT,
    GPSIMD xv/eb-w0/memsets.  PSUM: 2 banks QK+V, 4 banks scores, 1 bank
    AV+transpose, 1 bank projection - the head of octet o+1 shares nothing
    with the tail of octet o.  Inputs ride the SP DGE ring (one fused DMA
    per tensor per octet), the single output DMA rides the ACT ring.
"""

import os
import sys

import numpy as np
import ml_dtypes

if "/opt/trn_rl_repo" not in sys.path:
    sys.path.insert(0, "/opt/trn_rl_repo")

P = 49          # tokens per window
NH = 16         # heads
HD = 32         # head dim
C = 512         # model dim
NCORES = 8
B_TOTAL = 2048
B_CORE = B_TOTAL // NCORES        # 256 windows per core
TOK = B_CORE * P                  # 12544 tokens per core
OCT = 32                          # octets (8 windows) per core
OCT_TOK = 8 * P                   # 392 tokens per octet
OCT_W = OCT_TOK + 16              # octet tile width incl 16-token overlap
OCT_W8 = OCT_TOK + 24             # fp8 tile width (dim-1 step must be %16)
TOK_PAD = TOK + 16
TOK_PAD8 = TOK + 32
BF16 = ml_dtypes.bfloat16
FP8 = ml_dtypes.float8_e4m3fn

USE_FP8 = True
SQ = 256.0     # fp8 scale folded into q weights (power of 2)
SK = 64.0      # fp8 scale folded into k weights
ESC = 1.0 / (SQ * SK) if USE_FP8 else 1.0


def _build(nc, has_bqk, has_bv, has_bp, n_oct=OCT):
    import concourse.bass as bass
    import concourse.mybir as mybir
    from concourse.tile import TileContext
    from concourse.masks import make_identity

    F32 = mybir.dt.float32
    BF = mybir.dt.bfloat16
    F8 = mybir.dt.float8e4
    Exp = mybir.ActivationFunctionType.Exp
    DR = mybir.MatmulPerfMode.DoubleRow

    if USE_FP8:
        x8 = nc.dram_tensor("x8", [128, 2, 2, TOK_PAD8], F8, kind="ExternalInput")
        w8 = nc.dram_tensor("w8", [2, 128, 2, 2 * C], F8, kind="ExternalInput")
    else:
        wqk = nc.dram_tensor("wqk", [C, 2 * C], BF, kind="ExternalInput")
    xT = nc.dram_tensor("xT", [C, TOK_PAD], BF, kind="ExternalInput")
    wv = nc.dram_tensor("wv", [C, C], BF, kind="ExternalInput")
    wp = nc.dram_tensor("wp", [C, C], BF, kind="ExternalInput")
    eb = nc.dram_tensor("eb", [128, 4, 4, P], BF, kind="ExternalInput")
    bqk = bv = bp = None
    if has_bqk:
        bqk = nc.dram_tensor("bqk", [1, 2 * C], BF, kind="ExternalInput")
    if has_bv:
        bv = nc.dram_tensor("bv", [1, C], BF, kind="ExternalInput")
    if has_bp:
        bp = nc.dram_tensor("bp", [1, C], BF, kind="ExternalInput")
    out = nc.dram_tensor("out", [C, TOK], BF, kind="ExternalOutput")

    def bcast_last(ap, n):
        return bass.AP(ap.tensor, ap.offset, [*ap.ap, [0, n]])

    with TileContext(nc) as tc:
        with (
            tc.tile_pool(name="singles", bufs=1) as singles,
            tc.tile_pool(name="xin", bufs=2) as x_pool,
            tc.tile_pool(name="qk", bufs=2) as qk_pool,
            tc.tile_pool(name="vsb", bufs=6) as v_pool,
            tc.tile_pool(name="se", bufs=3) as se_pool,
            tc.tile_pool(name="zr", bufs=8) as zr_pool,
            tc.tile_pool(name="attn", bufs=6) as attn_pool,
            tc.tile_pool(name="att", bufs=3) as atT_pool,
            tc.tile_pool(name="osb", bufs=4) as out_pool,
            tc.tile_pool(name="ps_head", bufs=2, space="PSUM") as ps_head,
            tc.tile_pool(name="ps_st", bufs=1, space="PSUM") as ps_st,
            tc.tile_pool(name="ps_avt", bufs=1, space="PSUM") as ps_avt,
            tc.tile_pool(name="ps_po", bufs=1, space="PSUM") as ps_po,
        ):
            # --- constants / weights ---
            if USE_FP8:
                w8_sb = []
                for ci in range(2):
                    w8_t = singles.tile([128, 2, 2 * C], F8, name=f"w8{ci}")
                    nc.sync.dma_start(out=w8_t, in_=w8[ci, :, :, :])
                    w8_sb.append(w8_t)
            else:
                wqk_sb = []
                for ci in range(4):
                    wqk_t = singles.tile([128, 2 * C], BF, name=f"wqk{ci}")
                    nc.sync.dma_start(out=wqk_t, in_=wqk[128 * ci:128 * (ci + 1), :])
                    wqk_sb.append(wqk_t)
            wv_sb = []
            wp_sb = []
            for ci in range(4):
                wv_t = singles.tile([128, C], BF, name=f"wv{ci}")
                nc.sync.dma_start(out=wv_t, in_=wv[128 * ci:128 * (ci + 1), :])
                wv_sb.append(wv_t)
                wp_t = singles.tile([128, C], BF, name=f"wp{ci}")
                nc.sync.dma_start(out=wp_t, in_=wp[128 * ci:128 * (ci + 1), :])
                wp_sb.append(wp_t)
            eb_sb = singles.tile([128, 4, 4, P], BF, name="ebsb")
            nc.sync.dma_start(out=eb_sb, in_=eb[:, :, :, :])
            ident = singles.tile([128, 128], BF, name="ident")
            make_identity(nc, ident)
            bqk_sb = bv_sb = bpT_sb = ones_row = None
            if has_bqk or has_bv or has_bp:
                ones_row = singles.tile([1, OCT_W], BF, name="onesrow")
                nc.vector.memset(ones_row, 1.0)
            if has_bqk:
                bqk_sb = singles.tile([1, 2 * C], BF, name="bqksb")
                nc.sync.dma_start(out=bqk_sb, in_=bqk[:, :])
            if has_bv:
                bv_sb = singles.tile([1, C], BF, name="bvsb")
                nc.sync.dma_start(out=bv_sb, in_=bv[:, :])
            if has_bp:
                bpT_sb = singles.tile([1, C], BF, name="bpsb")
                nc.sync.dma_start(out=bpT_sb, in_=bp[:, :])

            # pre-zero the block-diagonal ses tiles (exp/eb rewrite only the
            # two diagonal blocks; the zero padding is what makes the fused
            # 2-window AV stationary sound) and pre-set the v denominator
            # ones column (the v copy writes only [:, :, 0:32]).
            for b in range(3):
                sez = se_pool.tile([128, 4, 4, 128], BF, name=f"sez{b}",
                                   tag="se")
                nc.gpsimd.memset(sez, 0.0)
            for b in range(6):
                vz = v_pool.tile([128, NH, 33], BF, name=f"vz{b}", tag="v")
                nc.gpsimd.memset(vz, 1.0)

            # --- main loop ---
            for o in range(n_oct):
                t0 = o * OCT_TOK
                # input DMAs (SP ring only carries inputs; outputs go via
                # the Activation ring so next-octet loads are never queued
                # behind this octet's stores)
                if USE_FP8:
                    x8t = x_pool.tile([128, 2, 2, OCT_W8], F8,
                                      name=f"x8{o}", tag="x8")
                    x8a = x8[:, :, :, :]
                    src8 = bass.AP(
                        x8a.tensor, x8a.offset + t0,
                        [[4 * TOK_PAD8, 128], [TOK_PAD8, 4], [1, OCT_W8]])
                    nc.sync.dma_start(out=x8t, in_=src8)
                xtile = x_pool.tile([128, 4, OCT_W], BF, name=f"xt{o}",
                                    tag="xt")
                xTa = xT[:, :]
                srcx = bass.AP(
                    xTa.tensor, xTa.offset + t0,
                    [[TOK_PAD, 128], [128 * TOK_PAD, 4], [1, OCT_W]])
                nc.sync.dma_start(out=xtile, in_=srcx)
                xts = [xtile[:, ci, :] for ci in range(4)]
                # slot-expanded copies for the V stationary (weights APs may
                # only have one free dim); SBUF->SBUF so Pool can do them
                xvs = []
                for ci in range(4):
                    xv = x_pool.tile([128, 4, 128], BF, name=f"xv{o}_{ci}",
                                     tag=f"xv{ci}")
                    xr = xts[ci]
                    src = bass.AP(xr.tensor, xr.offset,
                                  [xr.ap[0], [98, 4], [P, 2], [1, 64]])
                    nc.gpsimd.tensor_copy(xv, src)
                    xvs.append(xv)

                # QK projection: q tiles feature-major [128, OCT_W]; k tiles
                # slot-expanded [128, 4pair, 128] for contiguous stationaries.
                # ft order interleaves q/k so score matmuls unblock early.
                qs = [None] * 4
                ks = [None] * 4
                for ft in (0, 4, 1, 5, 2, 6, 3, 7):
                    ps = ps_head.tile([128, OCT_W], F32, name=f"qkp{o}_{ft}",
                                      tag="head")
                    if USE_FP8:
                        for ci in range(2):
                            nc.tensor.matmul(
                                ps, w8_sb[ci][:, :, 128 * ft:128 * (ft + 1)],
                                x8t[:, ci, :, 0:OCT_W],
                                start=(ci == 0),
                                stop=(ci == 1 and not has_bqk),
                                perf_mode=DR)
                    else:
                        for ci in range(4):
                            nc.tensor.matmul(
                                ps, wqk_sb[ci][:, 128 * ft:128 * (ft + 1)],
                                xts[ci], start=(ci == 0),
                                stop=(ci == 3 and not has_bqk))
                    if has_bqk:
                        nc.tensor.matmul(ps, bqk_sb[:, 128 * ft:128 * (ft + 1)],
                                         ones_row, start=False, stop=True)
                    if ft < 4:
                        q_sb = qk_pool.tile([128, OCT_TOK], BF, name=f"q{o}_{ft}",
                                            tag=f"q{ft}")
                        nc.scalar.copy(q_sb, ps[:, 0:OCT_TOK])
                        qs[ft] = q_sb
                    else:
                        k_sb = qk_pool.tile([128, 4, 128], BF, name=f"k{o}_{ft}",
                                            tag=f"k{ft}")
                        src = bass.AP(ps.tensor, ps.offset,
                                      [ps.ap[0], [98, 4], [P, 2], [1, 64]])
                        nc.vector.tensor_copy(k_sb, src)
                        ks[ft - 4] = k_sb

                # V projection (all pairs; slot-expanded stationary)
                v_sbs = []
                for p in range(4):
                    vps = ps_head.tile([128, C], F32, name=f"vp{o}_{p}",
                                       tag="head")
                    for ci in range(4):
                        nc.tensor.matmul(
                            vps, xvs[ci][:, p, :],
                            wv_sb[ci], start=(ci == 0),
                            stop=(ci == 3 and not has_bv))
                    if has_bv:
                        nc.tensor.matmul(vps, ones_row[:, 0:128], bv_sb,
                                         start=False, stop=True)
                    v_sb = v_pool.tile([128, NH, 33], BF, name=f"v{o}_{p}",
                                       tag="v")
                    vv = vps.rearrange("q (h d) -> q h d", h=NH)
                    if p >= 1:
                        nc.vector.tensor_copy(v_sb[:, :, 0:32], vv)
                    else:
                        nc.scalar.copy(v_sb[:, :, 0:32], vv)
                    v_sbs.append(v_sb)

                # scores: one matmul per (pair, head) covering both windows,
                # all 16 into one 4-bank PSUM tile (j selects the bank, so
                # each tile_position row group owns its own bank); row groups
                # rotate (j innermost) so LDWEIGHTS overlaps
                for p in range(4):
                    pt0 = 98 * p
                    stp = ps_st.tile([128, 4, 4, 128], F32, name=f"st{o}_{p}",
                                     tag="st")
                    sesp = se_pool.tile([128, 4, 4, 128], BF,
                                        name=f"se{o}_{p}", tag="se")
                    for i in range(4):
                        for j in range(4):
                            r = 32 * j
                            nc.tensor.matmul(
                                stp[:, j, i, 0:98],
                                ks[i][r:r + 32, p, :],
                                qs[i][r:r + 32, pt0:pt0 + 98],
                                start=True, stop=True,
                                tile_position=(r, 0))
                    with tc.high_priority():
                        nc.scalar.activation(
                            out=sesp[0:P, :, :, 0:P], in_=stp[0:P, :, :, 0:P],
                            func=Exp, scale=ESC)
                        nc.scalar.activation(
                            out=sesp[64:64 + P, :, :, 64:64 + P],
                            in_=stp[64:64 + P, :, :, P:2 * P],
                            func=Exp, scale=ESC)
                    # multiplicative rel-pos bias on the two diagonal
                    # blocks; j-halves so AV g=0 (j 0,1) releases early
                    for jh in range(2):
                        js = slice(2 * jh, 2 * jh + 2)
                        nc.gpsimd.tensor_mul(sesp[0:P, js, :, 0:P],
                                             sesp[0:P, js, :, 0:P],
                                             eb_sb[0:P, js, :, :])
                        nc.vector.tensor_mul(sesp[64:64 + P, js, :, 64:64 + P],
                                             sesp[64:64 + P, js, :, 64:64 + P],
                                             eb_sb[64:64 + P, js, :, :])

                    # AV: one matmul per head over the full 128-partition
                    # block-diagonal stationary; 33rd v column accumulates
                    # the softmax denominator
                    v_sb = v_sbs[p]
                    attn_sb = attn_pool.tile([128, NH, HD], BF,
                                             name=f"attn{o}_{p}", tag="attn")
                    for g in range(2):
                        # heads h = 4i + 2g + jj; one avt bank cycles
                        # av(g=0) -> av(g=1) -> atp within each pair
                        av = ps_avt.tile([128, 4, 2, 33], F32,
                                         name=f"av{o}_{p}_{g}", tag="avt")
                        for i in range(4):
                            for jj in range(2):
                                h = 4 * i + 2 * g + jj
                                nc.tensor.matmul(av[:, i, jj, :],
                                                 sesp[:, 2 * g + jj, i, :],
                                                 v_sb[:, h, :],
                                                 start=True, stop=True)
                        zr = zr_pool.tile([128, 4, 2], F32,
                                          name=f"zr{o}_{p}_{g}", tag="zr")
                        with tc.high_priority():
                            nc.vector.reciprocal(zr, av[:, :, :, 32])
                            attn_v = bass.AP(attn_sb.tensor,
                                             attn_sb.offset + 64 * g,
                                             [attn_sb.ap[0], [128, 4], [32, 2],
                                              [1, HD]])
                            nc.vector.tensor_mul(attn_v, av[:, :, :, 0:32],
                                                 bcast_last(zr, HD))

                    # transpose to feature-major (po bank; av recycles sooner)
                    atp = ps_po.tile([128, 4, 128], BF, name=f"atp{o}_{p}",
                                     tag="po")
                    attn_flat = attn_sb.rearrange("q h d -> q (h d)")
                    for ci in range(4):
                        nc.tensor.transpose(atp[:, ci, :],
                                            attn_flat[:, 128 * ci:128 * (ci + 1)],
                                            ident)
                    if p == 0:
                        atT = atT_pool.tile([128, 4, 4, 128], BF,
                                            name=f"atT{o}", tag="atT")
                    dst = bass.AP(atT.tensor, atT.offset + 128 * p,
                                  [atT.ap[0], [512, 4], [1, 128]])
                    with tc.high_priority():
                        if p % 2:
                            nc.scalar.copy(dst, atp)
                        else:
                            nc.vector.tensor_copy(dst, atp)

                # output projection, feature-major; the moving AP gathers the
                # 49-token windows out of the 64-token slots (dense 392 cols)
                osb = out_pool.tile([128, 4, OCT_TOK], BF, name=f"o{o}",
                                    tag="osb")
                for et in range(4):
                    poT = ps_po.tile([128, OCT_TOK], F32, name=f"po{o}_{et}",
                                     tag="po")
                    for ci in range(4):
                        rhs = bass.AP(atT.tensor, atT.offset + 512 * ci,
                                      [atT.ap[0], [128, 4], [64, 2], [1, P]])
                        nc.tensor.matmul(poT, wp_sb[ci][:, 128 * et:128 * (et + 1)],
                                         rhs, start=(ci == 0),
                                         stop=(ci == 3 and not has_bp))
                    if has_bp:
                        nc.tensor.matmul(poT, bpT_sb[:, 128 * et:128 * (et + 1)],
                                         ones_row[:, 0:OCT_TOK],
                                         start=False, stop=True)
                    if et < 2:
                        nc.scalar.copy(osb[:, et, :], poT)
                    else:
                        nc.vector.tensor_copy(osb[:, et, :], poT)
                # single store per octet on the Activation DGE ring
                outa = out[:, :]
                dst = bass.AP(outa.tensor, outa.offset + t0,
                              [[TOK, 128], [128 * TOK, 4], [1, OCT_TOK]])
                nc.scalar.dma_start(out=dst, in_=osb)
    return nc


def _host_prep(x, qkv_w, qkv_b, proj_w, proj_b, rpb_table, rel_index):
    scale = HD ** -0.5
    # weights: qkv feature order is (3, NH, HD) -> q=0:512, k=512:1024, v=1024:1536
    wq = qkv_w[0:C, :] * scale          # fold attention scale into q
    wk = qkv_w[C:2 * C, :]
    wv = qkv_w[2 * C:3 * C, :]
    wv_t = np.ascontiguousarray(wv.T).astype(BF16)              # [C, C]
    wp_t = np.ascontiguousarray(proj_w.T).astype(BF16)          # [C, C]

    bq = qkv_b[0:C] * scale
    bk = qkv_b[C:2 * C]
    if USE_FP8:
        wqk = np.concatenate([wq.T * SQ, wk.T * SK], axis=1)    # [C, 2C]
        # feature f = ci*256 + i*128 + p  ->  w8[ci][p, i, m]
        w8 = np.ascontiguousarray(
            wqk.reshape(2, 2, 128, 2 * C).transpose(0, 2, 1, 3)).astype(FP8)
        bqk = np.concatenate([bq * SQ, bk * SK])[None, :].astype(BF16)
    else:
        wqk = np.concatenate([wq.T, wk.T], axis=1).astype(BF16)
        bqk = np.concatenate([bq, bk])[None, :].astype(BF16)

    bias = rpb_table[rel_index]                  # [n, m, NH]
    biasT = np.transpose(bias, (2, 1, 0))        # [h, m, n]
    ebias = np.exp(biasT.astype(np.float64)).astype(np.float32)
    ebp = np.ones((128, NH, P), np.float32)
    ebp[0:P] = np.transpose(ebias, (1, 0, 2))    # rows 0:49  (window slot 0)
    ebp[64:64 + P] = ebp[0:P]                    # rows 64:113 (window slot 1)
    # regroup heads: [128, j, i, P] with head 4*i + j at [:, j, i, :]
    eb = np.ascontiguousarray(
        ebp.reshape(128, 4, 4, P).transpose(0, 2, 1, 3)).astype(BF16)

    bv_ = qkv_b[2 * C:3 * C]
    bv = bv_[None, :].astype(BF16)
    bp = proj_b[None, :].astype(BF16)

    has_bqk = bool(np.any(qkv_b[0:2 * C] != 0))
    has_bv = bool(np.any(bv_ != 0))
    has_bp = bool(np.any(proj_b != 0))

    in_maps = []
    for c in range(NCORES):
        xc = np.asarray(x[c * B_CORE:(c + 1) * B_CORE]).reshape(TOK, C)
        xTc = np.zeros((C, TOK_PAD), np.float32)
        xTc[:, :TOK] = xc.T
        m = {"xT": xTc.astype(BF16), "wv": wv_t, "wp": wp_t, "eb": eb}
        if USE_FP8:
            x8c = np.zeros((C, TOK_PAD8), np.float32)
            x8c[:, :TOK] = xc.T
            m["x8"] = np.ascontiguousarray(
                x8c.reshape(2, 2, 128, TOK_PAD8).transpose(2, 0, 1, 3)).astype(FP8)
            m["w8"] = w8
        else:
            m["wqk"] = wqk
        if has_bqk:
            m["bqk"] = bqk
        if has_bv:
            m["bv"] = bv
        if has_bp:
            m["bp"] = bp
        in_maps.append(m)
    return in_maps, has_bqk, has_bv, has_bp


def kernel(x, qkv_w, qkv_b, proj_w, proj_b, rpb_table, rel_index):
    from concourse import bacc
    from concourse.bass_utils import run_bass_kernel_spmd

    in_maps, has_bqk, has_bv, has_bp = _host_prep(
        np.asarray(x, np.float32), np.asarray(qkv_w, np.float32),
        np.asarray(qkv_b, np.float32), np.asarray(proj_w, np.float32),
        np.asarray(proj_b, np.float32), np.asarray(rpb_table, np.float32),
        np.asarray(rel_index))

    nc = bacc.Bacc()
    _build(nc, has_bqk, has_bv, has_bp)
    nc.finalize()

    trace = os.environ.get("BASS_KERNEL_TRACE", "") == "1"
    res = run_bass_kernel_spmd(nc, in_maps, core_ids=list(range(NCORES)),
                               trace=trace)
    if trace and res.exec_time_ns is not None:
        print(f"HW exec time: {res.exec_time_ns} ns", flush=True)

    outs = [r["out"].astype(np.float32).T.reshape(B_CORE, P, C)
            for r in res.results]
    return np.concatenate(outs, axis=0)


# revision 7
# speedup vs baseline: 1.2789x; 1.0038x over previous
"""Swin windowed attention (B_=2048 windows, N=49 tokens, C=512, 16 heads)
on 8 Trainium2 NeuronCores, data-parallel over windows (256 windows/core).

Per-core layout (v3; ~1.4x fewer PE instructions and ~35%% less engine work
than the v1 baseline, with input/output DMA rings decoupled):

  - q,k projection runs in fp8e4 with DoubleRow perf mode (K=256/pass, 2
    passes): 16 matmuls/octet instead of 32.  Host folds the attention
    scale and power-of-2 fp8 scales (q x256, k x64) into the weights; the
    combined descale rides the exp activation's scale operand for free.
  - scores^T are computed two-windows-per-matmul: the stationary is a
    [32, 128] slice of a "kslot" tile whose columns are the pair's two
    windows at 64-aligned slots (materialized contiguously so FWL stays
    on); the moving q covers both windows (98 cols).  64 score matmuls
    per octet at 4 rotating row groups (LDWEIGHTS overlaps), all 16 of a
    pair accumulating into one 4-bank PSUM tile (bank = tile_position row).
  - exp'd scores land in zero-padded block-diagonal "sesp" tiles
    [128, 4j, 4i, 128] (w0 at [0:49, ..., 0:49], w1 at [64:113, ..., 64:113],
    zeros memset once per pool buffer).  One AV matmul per (pair, head)
    contracts all 128 partitions with the contiguous [128, 128] stationary:
    64 AV matmuls/octet; a 33rd ones-column of v accumulates the softmax
    denominator.
  - v projection uses slot-expanded x copies produced on the otherwise-idle
    GPSIMD engine (SBUF->SBUF; PSUM is GPSIMD-inaccessible, and matmul
    weights APs only allow one free dim).
  - normalization = DVE reciprocal + free-broadcast multiply; attention
    output is PE-transposed to feature-major, and the output projection is
    emitted feature-major with the moving AP gathering 49-token windows out
    of the 64-token slots; out^T [C, TOK] is un-transposed on the host.
  - engine budget (balance tuned against the TimelineSim cost model): ACT
    exp/q-copies + shares of v/atT/osb; DVE kslot/eb-w1/recip/attn + shares
    of v/atT/osb; GPSIMD xv/eb-w0/memsets.  PSUM: 2 banks QK+V, 4 banks
    scores (bank = tile_position row), 1 bank AV, 1 bank transpose+proj -
    the head of octet o+1 shares nothing with the tail of octet o.  Inputs
    ride the SP DGE ring (one fused DMA per tensor per octet, high
    priority), the single output DMA rides the ACT ring.
"""

import os
import sys

import numpy as np
import ml_dtypes

if "/opt/trn_rl_repo" not in sys.path:
    sys.path.insert(0, "/opt/trn_rl_repo")

P = 49          # tokens per window
NH = 16         # heads
HD = 32         # head dim
C = 512         # model dim
NCORES = 8
B_TOTAL = 2048
B_CORE = B_TOTAL // NCORES        # 256 windows per core
TOK = B_CORE * P                  # 12544 tokens per core
OCT = 32                          # octets (8 windows) per core
OCT_TOK = 8 * P                   # 392 tokens per octet
OCT_W = OCT_TOK + 16              # octet tile width incl 16-token overlap
OCT_W8 = OCT_TOK + 24             # fp8 tile width (dim-1 step must be %16)
TOK_PAD = TOK + 16
TOK_PAD8 = TOK + 32
BF16 = ml_dtypes.bfloat16
FP8 = ml_dtypes.float8_e4m3fn

USE_FP8 = True
SQ = 256.0     # fp8 scale folded into q weights (power of 2)
SK = 64.0      # fp8 scale folded into k weights
ESC = 1.0 / (SQ * SK) if USE_FP8 else 1.0


def _build(nc, has_bqk, has_bv, has_bp, n_oct=OCT):
    import concourse.bass as bass
    import concourse.mybir as mybir
    from concourse.tile import TileContext
    from concourse.masks import make_identity

    F32 = mybir.dt.float32
    BF = mybir.dt.bfloat16
    F8 = mybir.dt.float8e4
    Exp = mybir.ActivationFunctionType.Exp
    DR = mybir.MatmulPerfMode.DoubleRow

    if USE_FP8:
        x8 = nc.dram_tensor("x8", [128, 2, 2, TOK_PAD8], F8, kind="ExternalInput")
        w8 = nc.dram_tensor("w8", [2, 128, 2, 2 * C], F8, kind="ExternalInput")
    else:
        wqk = nc.dram_tensor("wqk", [C, 2 * C], BF, kind="ExternalInput")
    xT = nc.dram_tensor("xT", [C, TOK_PAD], BF, kind="ExternalInput")
    wv = nc.dram_tensor("wv", [C, C], BF, kind="ExternalInput")
    wp = nc.dram_tensor("wp", [C, C], BF, kind="ExternalInput")
    eb = nc.dram_tensor("eb", [128, 4, 4, P], BF, kind="ExternalInput")
    bqk = bv = bp = None
    if has_bqk:
        bqk = nc.dram_tensor("bqk", [1, 2 * C], BF, kind="ExternalInput")
    if has_bv:
        bv = nc.dram_tensor("bv", [1, C], BF, kind="ExternalInput")
    if has_bp:
        bp = nc.dram_tensor("bp", [1, C], BF, kind="ExternalInput")
    out = nc.dram_tensor("out", [C, TOK], BF, kind="ExternalOutput")

    def bcast_last(ap, n):
        return bass.AP(ap.tensor, ap.offset, [*ap.ap, [0, n]])

    with TileContext(nc) as tc:
        with (
            tc.tile_pool(name="singles", bufs=1) as singles,
            tc.tile_pool(name="xin", bufs=2) as x_pool,
            tc.tile_pool(name="qk", bufs=2) as qk_pool,
            tc.tile_pool(name="vsb", bufs=6) as v_pool,
            tc.tile_pool(name="se", bufs=3) as se_pool,
            tc.tile_pool(name="zr", bufs=8) as zr_pool,
            tc.tile_pool(name="attn", bufs=6) as attn_pool,
            tc.tile_pool(name="att", bufs=3) as atT_pool,
            tc.tile_pool(name="osb", bufs=4) as out_pool,
            tc.tile_pool(name="ps_head", bufs=2, space="PSUM") as ps_head,
            tc.tile_pool(name="ps_st", bufs=1, space="PSUM") as ps_st,
            tc.tile_pool(name="ps_avt", bufs=1, space="PSUM") as ps_avt,
            tc.tile_pool(name="ps_po", bufs=1, space="PSUM") as ps_po,
        ):
            # --- constants / weights ---
            if USE_FP8:
                w8_sb = []
                for ci in range(2):
                    w8_t = singles.tile([128, 2, 2 * C], F8, name=f"w8{ci}")
                    nc.sync.dma_start(out=w8_t, in_=w8[ci, :, :, :])
                    w8_sb.append(w8_t)
            else:
                wqk_sb = []
                for ci in range(4):
                    wqk_t = singles.tile([128, 2 * C], BF, name=f"wqk{ci}")
                    nc.sync.dma_start(out=wqk_t, in_=wqk[128 * ci:128 * (ci + 1), :])
                    wqk_sb.append(wqk_t)
            wv_sb = []
            wp_sb = []
            for ci in range(4):
                wv_t = singles.tile([128, C], BF, name=f"wv{ci}")
                nc.sync.dma_start(out=wv_t, in_=wv[128 * ci:128 * (ci + 1), :])
                wv_sb.append(wv_t)
                wp_t = singles.tile([128, C], BF, name=f"wp{ci}")
                nc.sync.dma_start(out=wp_t, in_=wp[128 * ci:128 * (ci + 1), :])
                wp_sb.append(wp_t)
            eb_sb = singles.tile([128, 4, 4, P], BF, name="ebsb")
            nc.sync.dma_start(out=eb_sb, in_=eb[:, :, :, :])
            ident = singles.tile([128, 128], BF, name="ident")
            make_identity(nc, ident)
            bqk_sb = bv_sb = bpT_sb = ones_row = None
            if has_bqk or has_bv or has_bp:
                ones_row = singles.tile([1, OCT_W], BF, name="onesrow")
                nc.vector.memset(ones_row, 1.0)
            if has_bqk:
                bqk_sb = singles.tile([1, 2 * C], BF, name="bqksb")
                nc.sync.dma_start(out=bqk_sb, in_=bqk[:, :])
            if has_bv:
                bv_sb = singles.tile([1, C], BF, name="bvsb")
                nc.sync.dma_start(out=bv_sb, in_=bv[:, :])
            if has_bp:
                bpT_sb = singles.tile([1, C], BF, name="bpsb")
                nc.sync.dma_start(out=bpT_sb, in_=bp[:, :])

            # pre-zero the block-diagonal ses tiles (exp/eb rewrite only the
            # two diagonal blocks; the zero padding is what makes the fused
            # 2-window AV stationary sound) and pre-set the v denominator
            # ones column (the v copy writes only [:, :, 0:32]).
            for b in range(3):
                sez = se_pool.tile([128, 4, 4, 128], BF, name=f"sez{b}",
                                   tag="se")
                nc.gpsimd.memset(sez, 0.0)
            for b in range(6):
                vz = v_pool.tile([128, NH, 33], BF, name=f"vz{b}", tag="v")
                nc.gpsimd.memset(vz, 1.0)

            # --- main loop ---
            for o in range(n_oct):
                t0 = o * OCT_TOK
                # input DMAs (SP ring only carries inputs; outputs go via
                # the Activation ring so next-octet loads are never queued
                # behind this octet's stores)
                if USE_FP8:
                    x8t = x_pool.tile([128, 2, 2, OCT_W8], F8,
                                      name=f"x8{o}", tag="x8")
                    x8a = x8[:, :, :, :]
                    src8 = bass.AP(
                        x8a.tensor, x8a.offset + t0,
                        [[4 * TOK_PAD8, 128], [TOK_PAD8, 4], [1, OCT_W8]])
                    with tc.high_priority():
                        nc.sync.dma_start(out=x8t, in_=src8)
                xtile = x_pool.tile([128, 4, OCT_W], BF, name=f"xt{o}",
                                    tag="xt")
                xTa = xT[:, :]
                srcx = bass.AP(
                    xTa.tensor, xTa.offset + t0,
                    [[TOK_PAD, 128], [128 * TOK_PAD, 4], [1, OCT_W]])
                nc.sync.dma_start(out=xtile, in_=srcx)
                xts = [xtile[:, ci, :] for ci in range(4)]
                # slot-expanded copies for the V stationary (weights APs may
                # only have one free dim); SBUF->SBUF so Pool can do them
                xvs = []
                for ci in range(4):
                    xv = x_pool.tile([128, 4, 128], BF, name=f"xv{o}_{ci}",
                                     tag=f"xv{ci}")
                    xr = xts[ci]
                    src = bass.AP(xr.tensor, xr.offset,
                                  [xr.ap[0], [98, 4], [P, 2], [1, 64]])
                    nc.gpsimd.tensor_copy(xv, src)
                    xvs.append(xv)

                # QK projection: q tiles feature-major [128, OCT_W]; k tiles
                # slot-expanded [128, 4pair, 128] for contiguous stationaries.
                # ft order interleaves q/k so score matmuls unblock early.
                qs = [None] * 4
                ks = [None] * 4
                for ft in (0, 4, 1, 5, 2, 6, 3, 7):
                    ps = ps_head.tile([128, OCT_W], F32, name=f"qkp{o}_{ft}",
                                      tag="head")
                    if USE_FP8:
                        for ci in range(2):
                            nc.tensor.matmul(
                                ps, w8_sb[ci][:, :, 128 * ft:128 * (ft + 1)],
                                x8t[:, ci, :, 0:OCT_W],
                                start=(ci == 0),
                                stop=(ci == 1 and not has_bqk),
                                perf_mode=DR)
                    else:
                        for ci in range(4):
                            nc.tensor.matmul(
                                ps, wqk_sb[ci][:, 128 * ft:128 * (ft + 1)],
                                xts[ci], start=(ci == 0),
                                stop=(ci == 3 and not has_bqk))
                    if has_bqk:
                        nc.tensor.matmul(ps, bqk_sb[:, 128 * ft:128 * (ft + 1)],
                                         ones_row, start=False, stop=True)
                    if ft < 4:
                        q_sb = qk_pool.tile([128, OCT_TOK], BF, name=f"q{o}_{ft}",
                                            tag=f"q{ft}")
                        nc.scalar.copy(q_sb, ps[:, 0:OCT_TOK])
                        qs[ft] = q_sb
                    else:
                        k_sb = qk_pool.tile([128, 4, 128], BF, name=f"k{o}_{ft}",
                                            tag=f"k{ft}")
                        src = bass.AP(ps.tensor, ps.offset,
                                      [ps.ap[0], [98, 4], [P, 2], [1, 64]])
                        nc.vector.tensor_copy(k_sb, src)
                        ks[ft - 4] = k_sb

                # V projection (all pairs; slot-expanded stationary)
                v_sbs = []
                for p in range(4):
                    vps = ps_head.tile([128, C], F32, name=f"vp{o}_{p}",
                                       tag="head")
                    for ci in range(4):
                        nc.tensor.matmul(
                            vps, xvs[ci][:, p, :],
                            wv_sb[ci], start=(ci == 0),
                            stop=(ci == 3 and not has_bv))
                    if has_bv:
                        nc.tensor.matmul(vps, ones_row[:, 0:128], bv_sb,
                                         start=False, stop=True)
                    v_sb = v_pool.tile([128, NH, 33], BF, name=f"v{o}_{p}",
                                       tag="v")
                    vv = vps.rearrange("q (h d) -> q h d", h=NH)
                    if p >= 1:
                        nc.vector.tensor_copy(v_sb[:, :, 0:32], vv)
                    else:
                        nc.scalar.copy(v_sb[:, :, 0:32], vv)
                    v_sbs.append(v_sb)

                # scores: one matmul per (pair, head) covering both windows,
                # all 16 into one 4-bank PSUM tile (j selects the bank, so
                # each tile_position row group owns its own bank); row groups
                # rotate (j innermost) so LDWEIGHTS overlaps
                for p in range(4):
                    pt0 = 98 * p
                    stp = ps_st.tile([128, 4, 4, 128], F32, name=f"st{o}_{p}",
                                     tag="st")
                    sesp = se_pool.tile([128, 4, 4, 128], BF,
                                        name=f"se{o}_{p}", tag="se")
                    for i in range(4):
                        for j in range(4):
                            r = 32 * j
                            nc.tensor.matmul(
                                stp[:, j, i, 0:98],
                                ks[i][r:r + 32, p, :],
                                qs[i][r:r + 32, pt0:pt0 + 98],
                                start=True, stop=True,
                                tile_position=(r, 0))
                    with tc.high_priority():
                        nc.scalar.activation(
                            out=sesp[0:P, :, :, 0:P], in_=stp[0:P, :, :, 0:P],
                            func=Exp, scale=ESC)
                        nc.scalar.activation(
                            out=sesp[64:64 + P, :, :, 64:64 + P],
                            in_=stp[64:64 + P, :, :, P:2 * P],
                            func=Exp, scale=ESC)
                    # multiplicative rel-pos bias on the two diagonal
                    # blocks; j-halves so AV g=0 (j 0,1) releases early
                    for jh in range(2):
                        js = slice(2 * jh, 2 * jh + 2)
                        nc.gpsimd.tensor_mul(sesp[0:P, js, :, 0:P],
                                             sesp[0:P, js, :, 0:P],
                                             eb_sb[0:P, js, :, :])
                        nc.vector.tensor_mul(sesp[64:64 + P, js, :, 64:64 + P],
                                             sesp[64:64 + P, js, :, 64:64 + P],
                                             eb_sb[64:64 + P, js, :, :])

                    # AV: one matmul per head over the full 128-partition
                    # block-diagonal stationary; 33rd v column accumulates
                    # the softmax denominator
                    v_sb = v_sbs[p]
                    attn_sb = attn_pool.tile([128, NH, HD], BF,
                                             name=f"attn{o}_{p}", tag="attn")
                    for g in range(2):
                        # heads h = 4i + 2g + jj; one avt bank cycles
                        # av(g=0) -> av(g=1) -> atp within each pair
                        av = ps_avt.tile([128, 4, 2, 33], F32,
                                         name=f"av{o}_{p}_{g}", tag="avt")
                        for i in range(4):
                            for jj in range(2):
                                h = 4 * i + 2 * g + jj
                                nc.tensor.matmul(av[:, i, jj, :],
                                                 sesp[:, 2 * g + jj, i, :],
                                                 v_sb[:, h, :],
                                                 start=True, stop=True)
                        zr = zr_pool.tile([128, 4, 2], F32,
                                          name=f"zr{o}_{p}_{g}", tag="zr")
                        with tc.high_priority():
                            nc.vector.reciprocal(zr, av[:, :, :, 32])
                            attn_v = bass.AP(attn_sb.tensor,
                                             attn_sb.offset + 64 * g,
                                             [attn_sb.ap[0], [128, 4], [32, 2],
                                              [1, HD]])
                            nc.vector.tensor_mul(attn_v, av[:, :, :, 0:32],
                                                 bcast_last(zr, HD))

                    # transpose to feature-major (po bank; av recycles sooner)
                    atp = ps_po.tile([128, 4, 128], BF, name=f"atp{o}_{p}",
                                     tag="po")
                    attn_flat = attn_sb.rearrange("q h d -> q (h d)")
                    for ci in range(4):
                        nc.tensor.transpose(atp[:, ci, :],
                                            attn_flat[:, 128 * ci:128 * (ci + 1)],
                                            ident)
                    if p == 0:
                        atT = atT_pool.tile([128, 4, 4, 128], BF,
                                            name=f"atT{o}", tag="atT")
                    dst = bass.AP(atT.tensor, atT.offset + 128 * p,
                                  [atT.ap[0], [512, 4], [1, 128]])
                    with tc.high_priority():
                        if p % 2:
                            nc.scalar.copy(dst, atp)
                        else:
                            nc.vector.tensor_copy(dst, atp)

                # output projection, feature-major; the moving AP gathers the
                # 49-token windows out of the 64-token slots (dense 392 cols)
                osb = out_pool.tile([128, 4, OCT_TOK], BF, name=f"o{o}",
                                    tag="osb")
                for et in range(4):
                    poT = ps_po.tile([128, OCT_TOK], F32, name=f"po{o}_{et}",
                                     tag="po")
                    for ci in range(4):
                        rhs = bass.AP(atT.tensor, atT.offset + 512 * ci,
                                      [atT.ap[0], [128, 4], [64, 2], [1, P]])
                        nc.tensor.matmul(poT, wp_sb[ci][:, 128 * et:128 * (et + 1)],
                                         rhs, start=(ci == 0),
                                         stop=(ci == 3 and not has_bp))
                    if has_bp:
                        nc.tensor.matmul(poT, bpT_sb[:, 128 * et:128 * (et + 1)],
                                         ones_row[:, 0:OCT_TOK],
                                         start=False, stop=True)
                    if et < 2:
                        nc.scalar.copy(osb[:, et, :], poT)
                    else:
                        nc.vector.tensor_copy(osb[:, et, :], poT)
                # single store per octet on the Activation DGE ring
                outa = out[:, :]
                dst = bass.AP(outa.tensor, outa.offset + t0,
                              [[TOK, 128], [128 * TOK, 4], [1, OCT_TOK]])
                nc.scalar.dma_start(out=dst, in_=osb)
    return nc


def _host_prep(x, qkv_w, qkv_b, proj_w, proj_b, rpb_table, rel_index):
    scale = HD ** -0.5
    # weights: qkv feature order is (3, NH, HD) -> q=0:512, k=512:1024, v=1024:1536
    wq = qkv_w[0:C, :] * scale          # fold attention scale into q
    wk = qkv_w[C:2 * C, :]
    wv = qkv_w[2 * C:3 * C, :]
    wv_t = np.ascontiguousarray(wv.T).astype(BF16)              # [C, C]
    wp_t = np.ascontiguousarray(proj_w.T).astype(BF16)          # [C, C]

    bq = qkv_b[0:C] * scale
    bk = qkv_b[C:2 * C]
    if USE_FP8:
        wqk = np.concatenate([wq.T * SQ, wk.T * SK], axis=1)    # [C, 2C]
        # feature f = ci*256 + i*128 + p  ->  w8[ci][p, i, m]
        w8 = np.ascontiguousarray(
            wqk.reshape(2, 2, 128, 2 * C).transpose(0, 2, 1, 3)).astype(FP8)
        bqk = np.concatenate([bq * SQ, bk * SK])[None, :].astype(BF16)
    else:
        wqk = np.concatenate([wq.T, wk.T], axis=1).astype(BF16)
        bqk = np.concatenate([bq, bk])[None, :].astype(BF16)

    bias = rpb_table[rel_index]                  # [n, m, NH]
    biasT = np.transpose(bias, (2, 1, 0))        # [h, m, n]
    ebias = np.exp(biasT.astype(np.float64)).astype(np.float32)
    ebp = np.ones((128, NH, P), np.float32)
    ebp[0:P] = np.transpose(ebias, (1, 0, 2))    # rows 0:49  (window slot 0)
    ebp[64:64 + P] = ebp[0:P]                    # rows 64:113 (window slot 1)
    # regroup heads: [128, j, i, P] with head 4*i + j at [:, j, i, :]
    eb = np.ascontiguousarray(
        ebp.reshape(128, 4, 4, P).transpose(0, 2, 1, 3)).astype(BF16)

    bv_ = qkv_b[2 * C:3 * C]
    bv = bv_[None, :].astype(BF16)
    bp = proj_b[None, :].astype(BF16)

    has_bqk = bool(np.any(qkv_b[0:2 * C] != 0))
    has_bv = bool(np.any(bv_ != 0))
    has_bp = bool(np.any(proj_b != 0))

    in_maps = []
    for c in range(NCORES):
        xc = np.asarray(x[c * B_CORE:(c + 1) * B_CORE]).reshape(TOK, C)
        xTc = np.zeros((C, TOK_PAD), np.float32)
        xTc[:, :TOK] = xc.T
        m = {"xT": xTc.astype(BF16), "wv": wv_t, "wp": wp_t, "eb": eb}
        if USE_FP8:
            x8c = np.zeros((C, TOK_PAD8), np.float32)
            x8c[:, :TOK] = xc.T
            m["x8"] = np.ascontiguousarray(
                x8c.reshape(2, 2, 128, TOK_PAD8).transpose(2, 0, 1, 3)).astype(FP8)
            m["w8"] = w8
        else:
            m["wqk"] = wqk
        if has_bqk:
            m["bqk"] = bqk
        if has_bv:
            m["bv"] = bv
        if has_bp:
            m["bp"] = bp
        in_maps.append(m)
    return in_maps, has_bqk, has_bv, has_bp


def kernel(x, qkv_w, qkv_b, proj_w, proj_b, rpb_table, rel_index):
    from concourse import bacc
    from concourse.bass_utils import run_bass_kernel_spmd

    in_maps, has_bqk, has_bv, has_bp = _host_prep(
        np.asarray(x, np.float32), np.asarray(qkv_w, np.float32),
        np.asarray(qkv_b, np.float32), np.asarray(proj_w, np.float32),
        np.asarray(proj_b, np.float32), np.asarray(rpb_table, np.float32),
        np.asarray(rel_index))

    nc = bacc.Bacc()
    _build(nc, has_bqk, has_bv, has_bp)
    nc.finalize()

    trace = os.environ.get("BASS_KERNEL_TRACE", "") == "1"
    res = run_bass_kernel_spmd(nc, in_maps, core_ids=list(range(NCORES)),
                               trace=trace)
    if trace and res.exec_time_ns is not None:
        print(f"HW exec time: {res.exec_time_ns} ns", flush=True)

    outs = [r["out"].astype(np.float32).T.reshape(B_CORE, P, C)
            for r in res.results]
    return np.concatenate(outs, axis=0)


# revision 8
# speedup vs baseline: 1.2875x; 1.0067x over previous
"""Swin windowed attention (B_=2048 windows, N=49 tokens, C=512, 16 heads)
on 8 Trainium2 NeuronCores, data-parallel over windows (256 windows/core).

Per-core layout (v3; ~1.4x fewer PE instructions and ~35%% less engine work
than the v1 baseline, with input/output DMA rings decoupled):

  - q,k projection runs in fp8e4 with DoubleRow perf mode (K=256/pass, 2
    passes): 16 matmuls/octet instead of 32.  Host folds the attention
    scale and power-of-2 fp8 scales (q x256, k x64) into the weights; the
    combined descale rides the exp activation's scale operand for free.
  - scores^T are computed two-windows-per-matmul: the stationary is a
    [32, 128] slice of a "kslot" tile whose columns are the pair's two
    windows at 64-aligned slots (materialized contiguously so FWL stays
    on); the moving q covers both windows (98 cols).  64 score matmuls
    per octet at 4 rotating row groups (LDWEIGHTS overlaps), all 16 of a
    pair accumulating into one 4-bank PSUM tile (bank = tile_position row).
  - exp'd scores land in zero-padded block-diagonal "sesp" tiles
    [128, 4j, 4i, 128] (w0 at [0:49, ..., 0:49], w1 at [64:113, ..., 64:113],
    zeros memset once per pool buffer).  One AV matmul per (pair, head)
    contracts all 128 partitions with the contiguous [128, 128] stationary:
    64 AV matmuls/octet; a 33rd ones-column of v accumulates the softmax
    denominator.
  - v projection uses slot-expanded x copies produced on the otherwise-idle
    GPSIMD engine (SBUF->SBUF; PSUM is GPSIMD-inaccessible, and matmul
    weights APs only allow one free dim).
  - normalization = DVE reciprocal + free-broadcast multiply; attention
    output is PE-transposed to feature-major, and the output projection is
    emitted feature-major with the moving AP gathering 49-token windows out
    of the 64-token slots; out^T [C, TOK] is un-transposed on the host.
  - engine budget (balance tuned against the TimelineSim cost model): ACT
    exp/q-copies + shares of v/atT/osb; DVE kslot/eb-w1/recip/attn + shares
    of v/atT/osb; GPSIMD xv/eb-w0/memsets.  PSUM: 2 banks QK+V, 4 banks
    scores (bank = tile_position row), 1 bank AV, 1 bank transpose+proj -
    the head of octet o+1 shares nothing with the tail of octet o.  Inputs
    ride the SP DGE ring (one fused DMA per tensor per octet, high
    priority), the single output DMA rides the ACT ring.
"""

import os
import sys

import numpy as np
import ml_dtypes

if "/opt/trn_rl_repo" not in sys.path:
    sys.path.insert(0, "/opt/trn_rl_repo")

P = 49          # tokens per window
NH = 16         # heads
HD = 32         # head dim
C = 512         # model dim
NCORES = 8
B_TOTAL = 2048
B_CORE = B_TOTAL // NCORES        # 256 windows per core
TOK = B_CORE * P                  # 12544 tokens per core
OCT = 32                          # octets (8 windows) per core
OCT_TOK = 8 * P                   # 392 tokens per octet
OCT_W = OCT_TOK + 16              # octet tile width incl 16-token overlap
OCT_W8 = OCT_TOK + 24             # fp8 tile width (dim-1 step must be %16)
TOK_PAD = TOK + 16
TOK_PAD8 = TOK + 32
BF16 = ml_dtypes.bfloat16
FP8 = ml_dtypes.float8_e4m3fn

USE_FP8 = True
SQ = 256.0     # fp8 scale folded into q weights (power of 2)
SK = 64.0      # fp8 scale folded into k weights
ESC = 1.0 / (SQ * SK) if USE_FP8 else 1.0


def _build(nc, has_bqk, has_bv, has_bp, n_oct=OCT):
    import concourse.bass as bass
    import concourse.mybir as mybir
    from concourse.tile import TileContext
    from concourse.masks import make_identity

    F32 = mybir.dt.float32
    BF = mybir.dt.bfloat16
    F8 = mybir.dt.float8e4
    Exp = mybir.ActivationFunctionType.Exp
    DR = mybir.MatmulPerfMode.DoubleRow

    if USE_FP8:
        x8 = nc.dram_tensor("x8", [128, 2, 2, TOK_PAD8], F8, kind="ExternalInput")
        w8 = nc.dram_tensor("w8", [2, 128, 2, 2 * C], F8, kind="ExternalInput")
    else:
        wqk = nc.dram_tensor("wqk", [C, 2 * C], BF, kind="ExternalInput")
    xT = nc.dram_tensor("xT", [C, TOK_PAD], BF, kind="ExternalInput")
    wv = nc.dram_tensor("wv", [C, C], BF, kind="ExternalInput")
    wp = nc.dram_tensor("wp", [C, C], BF, kind="ExternalInput")
    eb = nc.dram_tensor("eb", [128, 4, 4, P], BF, kind="ExternalInput")
    bqk = bv = bp = None
    if has_bqk:
        bqk = nc.dram_tensor("bqk", [1, 2 * C], BF, kind="ExternalInput")
    if has_bv:
        bv = nc.dram_tensor("bv", [1, C], BF, kind="ExternalInput")
    if has_bp:
        bp = nc.dram_tensor("bp", [1, C], BF, kind="ExternalInput")
    out = nc.dram_tensor("out", [C, TOK], BF, kind="ExternalOutput")

    def bcast_last(ap, n):
        return bass.AP(ap.tensor, ap.offset, [*ap.ap, [0, n]])

    with TileContext(nc) as tc:
        with (
            tc.tile_pool(name="singles", bufs=1) as singles,
            tc.tile_pool(name="xin", bufs=2) as x_pool,
            tc.tile_pool(name="qk", bufs=2) as qk_pool,
            tc.tile_pool(name="vsb", bufs=6) as v_pool,
            tc.tile_pool(name="se", bufs=3) as se_pool,
            tc.tile_pool(name="zr", bufs=8) as zr_pool,
            tc.tile_pool(name="attn", bufs=2) as attn_pool,
            tc.tile_pool(name="att", bufs=3) as atT_pool,
            tc.tile_pool(name="osb", bufs=4) as out_pool,
            tc.tile_pool(name="ps_head", bufs=2, space="PSUM") as ps_head,
            tc.tile_pool(name="ps_st", bufs=1, space="PSUM") as ps_st,
            tc.tile_pool(name="ps_avt", bufs=1, space="PSUM") as ps_avt,
            tc.tile_pool(name="ps_po", bufs=1, space="PSUM") as ps_po,
        ):
            # --- constants / weights ---
            if USE_FP8:
                w8_sb = []
                for ci in range(2):
                    w8_t = singles.tile([128, 2, 2 * C], F8, name=f"w8{ci}")
                    nc.sync.dma_start(out=w8_t, in_=w8[ci, :, :, :])
                    w8_sb.append(w8_t)
            else:
                wqk_sb = []
                for ci in range(4):
                    wqk_t = singles.tile([128, 2 * C], BF, name=f"wqk{ci}")
                    nc.sync.dma_start(out=wqk_t, in_=wqk[128 * ci:128 * (ci + 1), :])
                    wqk_sb.append(wqk_t)
            wv_sb = []
            wp_sb = []
            for ci in range(4):
                wv_t = singles.tile([128, C], BF, name=f"wv{ci}")
                nc.sync.dma_start(out=wv_t, in_=wv[128 * ci:128 * (ci + 1), :])
                wv_sb.append(wv_t)
                wp_t = singles.tile([128, C], BF, name=f"wp{ci}")
                nc.sync.dma_start(out=wp_t, in_=wp[128 * ci:128 * (ci + 1), :])
                wp_sb.append(wp_t)
            eb_sb = singles.tile([128, 4, 4, P], BF, name="ebsb")
            nc.sync.dma_start(out=eb_sb, in_=eb[:, :, :, :])
            ident = singles.tile([128, 128], BF, name="ident")
            make_identity(nc, ident)
            bqk_sb = bv_sb = bpT_sb = ones_row = None
            if has_bqk or has_bv or has_bp:
                ones_row = singles.tile([1, OCT_W], BF, name="onesrow")
                nc.vector.memset(ones_row, 1.0)
            if has_bqk:
                bqk_sb = singles.tile([1, 2 * C], BF, name="bqksb")
                nc.sync.dma_start(out=bqk_sb, in_=bqk[:, :])
            if has_bv:
                bv_sb = singles.tile([1, C], BF, name="bvsb")
                nc.sync.dma_start(out=bv_sb, in_=bv[:, :])
            if has_bp:
                bpT_sb = singles.tile([1, C], BF, name="bpsb")
                nc.sync.dma_start(out=bpT_sb, in_=bp[:, :])

            # pre-zero the block-diagonal ses tiles (exp/eb rewrite only the
            # two diagonal blocks; the zero padding is what makes the fused
            # 2-window AV stationary sound) and pre-set the v denominator
            # ones column (the v copy writes only [:, :, 0:32]).
            for b in range(3):
                sez = se_pool.tile([128, 4, 4, 128], BF, name=f"sez{b}",
                                   tag="se")
                nc.gpsimd.memset(sez, 0.0)
            for b in range(6):
                vz = v_pool.tile([128, NH, 33], BF, name=f"vz{b}", tag="v")
                nc.gpsimd.memset(vz, 1.0)

            # --- main loop ---
            for o in range(n_oct):
                t0 = o * OCT_TOK
                # input DMAs (SP ring only carries inputs; outputs go via
                # the Activation ring so next-octet loads are never queued
                # behind this octet's stores)
                if USE_FP8:
                    x8t = x_pool.tile([128, 2, 2, OCT_W8], F8,
                                      name=f"x8{o}", tag="x8")
                    x8a = x8[:, :, :, :]
                    src8 = bass.AP(
                        x8a.tensor, x8a.offset + t0,
                        [[4 * TOK_PAD8, 128], [TOK_PAD8, 4], [1, OCT_W8]])
                    with tc.high_priority():
                        nc.sync.dma_start(out=x8t, in_=src8)
                xtile = x_pool.tile([128, 4, OCT_W], BF, name=f"xt{o}",
                                    tag="xt")
                xTa = xT[:, :]
                srcx = bass.AP(
                    xTa.tensor, xTa.offset + t0,
                    [[TOK_PAD, 128], [128 * TOK_PAD, 4], [1, OCT_W]])
                nc.sync.dma_start(out=xtile, in_=srcx)
                xts = [xtile[:, ci, :] for ci in range(4)]
                # slot-expanded copies for the V stationary (weights APs may
                # only have one free dim); SBUF->SBUF so Pool can do them
                xvs = []
                for ci in range(4):
                    xv = x_pool.tile([128, 4, 128], BF, name=f"xv{o}_{ci}",
                                     tag=f"xv{ci}")
                    xr = xts[ci]
                    src = bass.AP(xr.tensor, xr.offset,
                                  [xr.ap[0], [98, 4], [P, 2], [1, 64]])
                    with tc.high_priority():
                        nc.gpsimd.tensor_copy(xv, src)
                    xvs.append(xv)

                # QK projection: q tiles feature-major [128, OCT_W]; k tiles
                # slot-expanded [128, 4pair, 128] for contiguous stationaries.
                # ft order interleaves q/k so score matmuls unblock early.
                qs = [None] * 4
                ks = [None] * 4
                for ft in (0, 4, 1, 5, 2, 6, 3, 7):
                    ps = ps_head.tile([128, OCT_W], F32, name=f"qkp{o}_{ft}",
                                      tag="head")
                    if USE_FP8:
                        for ci in range(2):
                            nc.tensor.matmul(
                                ps, w8_sb[ci][:, :, 128 * ft:128 * (ft + 1)],
                                x8t[:, ci, :, 0:OCT_W],
                                start=(ci == 0),
                                stop=(ci == 1 and not has_bqk),
                                perf_mode=DR)
                    else:
                        for ci in range(4):
                            nc.tensor.matmul(
                                ps, wqk_sb[ci][:, 128 * ft:128 * (ft + 1)],
                                xts[ci], start=(ci == 0),
                                stop=(ci == 3 and not has_bqk))
                    if has_bqk:
                        nc.tensor.matmul(ps, bqk_sb[:, 128 * ft:128 * (ft + 1)],
                                         ones_row, start=False, stop=True)
                    if ft < 4:
                        q_sb = qk_pool.tile([128, OCT_TOK], BF, name=f"q{o}_{ft}",
                                            tag=f"q{ft}")
                        nc.scalar.copy(q_sb, ps[:, 0:OCT_TOK])
                        qs[ft] = q_sb
                    else:
                        k_sb = qk_pool.tile([128, 4, 128], BF, name=f"k{o}_{ft}",
                                            tag=f"k{ft}")
                        src = bass.AP(ps.tensor, ps.offset,
                                      [ps.ap[0], [98, 4], [P, 2], [1, 64]])
                        nc.vector.tensor_copy(k_sb, src)
                        ks[ft - 4] = k_sb

                # V projection (all pairs; slot-expanded stationary)
                v_sbs = []
                for p in range(4):
                    vps = ps_head.tile([128, C], F32, name=f"vp{o}_{p}",
                                       tag="head")
                    for ci in range(4):
                        nc.tensor.matmul(
                            vps, xvs[ci][:, p, :],
                            wv_sb[ci], start=(ci == 0),
                            stop=(ci == 3 and not has_bv))
                    if has_bv:
                        nc.tensor.matmul(vps, ones_row[:, 0:128], bv_sb,
                                         start=False, stop=True)
                    v_sb = v_pool.tile([128, NH, 33], BF, name=f"v{o}_{p}",
                                       tag="v")
                    vv = vps.rearrange("q (h d) -> q h d", h=NH)
                    if p >= 1:
                        nc.vector.tensor_copy(v_sb[:, :, 0:32], vv)
                    else:
                        nc.scalar.copy(v_sb[:, :, 0:32], vv)
                    v_sbs.append(v_sb)

                # scores: one matmul per (pair, head) covering both windows,
                # all 16 into one 4-bank PSUM tile (j selects the bank, so
                # each tile_position row group owns its own bank); row groups
                # rotate (j innermost) so LDWEIGHTS overlaps
                for p in range(4):
                    pt0 = 98 * p
                    stp = ps_st.tile([128, 4, 4, 128], F32, name=f"st{o}_{p}",
                                     tag="st")
                    sesp = se_pool.tile([128, 4, 4, 128], BF,
                                        name=f"se{o}_{p}", tag="se")
                    for i in range(4):
                        for j in range(4):
                            r = 32 * j
                            nc.tensor.matmul(
                                stp[:, j, i, 0:98],
                                ks[i][r:r + 32, p, :],
                                qs[i][r:r + 32, pt0:pt0 + 98],
                                start=True, stop=True,
                                tile_position=(r, 0))
                    with tc.high_priority():
                        nc.scalar.activation(
                            out=sesp[0:P, :, :, 0:P], in_=stp[0:P, :, :, 0:P],
                            func=Exp, scale=ESC)
                        nc.scalar.activation(
                            out=sesp[64:64 + P, :, :, 64:64 + P],
                            in_=stp[64:64 + P, :, :, P:2 * P],
                            func=Exp, scale=ESC)
                    # multiplicative rel-pos bias on the two diagonal
                    # blocks; j-halves so AV g=0 (j 0,1) releases early
                    for jh in range(2):
                        js = slice(2 * jh, 2 * jh + 2)
                        nc.gpsimd.tensor_mul(sesp[0:P, js, :, 0:P],
                                             sesp[0:P, js, :, 0:P],
                                             eb_sb[0:P, js, :, :])
                        nc.vector.tensor_mul(sesp[64:64 + P, js, :, 64:64 + P],
                                             sesp[64:64 + P, js, :, 64:64 + P],
                                             eb_sb[64:64 + P, js, :, :])

                    # AV: one matmul per head over the full 128-partition
                    # block-diagonal stationary; 33rd v column accumulates
                    # the softmax denominator
                    v_sb = v_sbs[p]
                    attn_sb = attn_pool.tile([128, NH, HD], BF,
                                             name=f"attn{o}_{p}", tag="attn")
                    for g in range(2):
                        # heads h = 4i + 2g + jj; one avt bank cycles
                        # av(g=0) -> av(g=1) -> atp within each pair
                        av = ps_avt.tile([128, 4, 2, 33], F32,
                                         name=f"av{o}_{p}_{g}", tag="avt")
                        for i in range(4):
                            for jj in range(2):
                                h = 4 * i + 2 * g + jj
                                nc.tensor.matmul(av[:, i, jj, :],
                                                 sesp[:, 2 * g + jj, i, :],
                                                 v_sb[:, h, :],
                                                 start=True, stop=True)
                        zr = zr_pool.tile([128, 4, 2], F32,
                                          name=f"zr{o}_{p}_{g}", tag="zr")
                        with tc.high_priority():
                            nc.vector.reciprocal(zr, av[:, :, :, 32])
                            attn_v = bass.AP(attn_sb.tensor,
                                             attn_sb.offset + 64 * g,
                                             [attn_sb.ap[0], [128, 4], [32, 2],
                                              [1, HD]])
                            nc.vector.tensor_mul(attn_v, av[:, :, :, 0:32],
                                                 bcast_last(zr, HD))

                    # transpose to feature-major (po bank; av recycles sooner)
                    atp = ps_po.tile([128, 4, 128], BF, name=f"atp{o}_{p}",
                                     tag="po")
                    attn_flat = attn_sb.rearrange("q h d -> q (h d)")
                    for ci in range(4):
                        nc.tensor.transpose(atp[:, ci, :],
                                            attn_flat[:, 128 * ci:128 * (ci + 1)],
                                            ident)
                    if p == 0:
                        atT = atT_pool.tile([128, 4, 4, 128], BF,
                                            name=f"atT{o}", tag="atT")
                    dst = bass.AP(atT.tensor, atT.offset + 128 * p,
                                  [atT.ap[0], [512, 4], [1, 128]])
                    with tc.high_priority():
                        if p % 2:
                            nc.scalar.copy(dst, atp)
                        else:
                            nc.vector.tensor_copy(dst, atp)

                # output projection, feature-major; the moving AP gathers the
                # 49-token windows out of the 64-token slots (dense 392 cols)
                osb = out_pool.tile([128, 4, OCT_TOK], BF, name=f"o{o}",
                                    tag="osb")
                for et in range(4):
                    poT = ps_po.tile([128, OCT_TOK], F32, name=f"po{o}_{et}",
                                     tag="po")
                    for ci in range(4):
                        rhs = bass.AP(atT.tensor, atT.offset + 512 * ci,
                                      [atT.ap[0], [128, 4], [64, 2], [1, P]])
                        nc.tensor.matmul(poT, wp_sb[ci][:, 128 * et:128 * (et + 1)],
                                         rhs, start=(ci == 0),
                                         stop=(ci == 3 and not has_bp))
                    if has_bp:
                        nc.tensor.matmul(poT, bpT_sb[:, 128 * et:128 * (et + 1)],
                                         ones_row[:, 0:OCT_TOK],
                                         start=False, stop=True)
                    if et < 2:
                        nc.scalar.copy(osb[:, et, :], poT)
                    else:
                        nc.vector.tensor_copy(osb[:, et, :], poT)
                # single store per octet on the Activation DGE ring
                outa = out[:, :]
                dst = bass.AP(outa.tensor, outa.offset + t0,
                              [[TOK, 128], [128 * TOK, 4], [1, OCT_TOK]])
                nc.scalar.dma_start(out=dst, in_=osb)
    return nc


def _host_prep(x, qkv_w, qkv_b, proj_w, proj_b, rpb_table, rel_index):
    scale = HD ** -0.5
    # weights: qkv feature order is (3, NH, HD) -> q=0:512, k=512:1024, v=1024:1536
    wq = qkv_w[0:C, :] * scale          # fold attention scale into q
    wk = qkv_w[C:2 * C, :]
    wv = qkv_w[2 * C:3 * C, :]
    wv_t = np.ascontiguousarray(wv.T).astype(BF16)              # [C, C]
    wp_t = np.ascontiguousarray(proj_w.T).astype(BF16)          # [C, C]

    bq = qkv_b[0:C] * scale
    bk = qkv_b[C:2 * C]
    if USE_FP8:
        wqk = np.concatenate([wq.T * SQ, wk.T * SK], axis=1)    # [C, 2C]
        # feature f = ci*256 + i*128 + p  ->  w8[ci][p, i, m]
        w8 = np.ascontiguousarray(
            wqk.reshape(2, 2, 128, 2 * C).transpose(0, 2, 1, 3)).astype(FP8)
        bqk = np.concatenate([bq * SQ, bk * SK])[None, :].astype(BF16)
    else:
        wqk = np.concatenate([wq.T, wk.T], axis=1).astype(BF16)
        bqk = np.concatenate([bq, bk])[None, :].astype(BF16)

    bias = rpb_table[rel_index]                  # [n, m, NH]
    biasT = np.transpose(bias, (2, 1, 0))        # [h, m, n]
    ebias = np.exp(biasT.astype(np.float64)).astype(np.float32)
    ebp = np.ones((128, NH, P), np.float32)
    ebp[0:P] = np.transpose(ebias, (1, 0, 2))    # rows 0:49  (window slot 0)
    ebp[64:64 + P] = ebp[0:P]                    # rows 64:113 (window slot 1)
    # regroup heads: [128, j, i, P] with head 4*i + j at [:, j, i, :]
    eb = np.ascontiguousarray(
        ebp.reshape(128, 4, 4, P).transpose(0, 2, 1, 3)).astype(BF16)

    bv_ = qkv_b[2 * C:3 * C]
    bv = bv_[None, :].astype(BF16)
    bp = proj_b[None, :].astype(BF16)

    has_bqk = bool(np.any(qkv_b[0:2 * C] != 0))
    has_bv = bool(np.any(bv_ != 0))
    has_bp = bool(np.any(proj_b != 0))

    in_maps = []
    for c in range(NCORES):
        xc = np.asarray(x[c * B_CORE:(c + 1) * B_CORE]).reshape(TOK, C)
        xTc = np.zeros((C, TOK_PAD), np.float32)
        xTc[:, :TOK] = xc.T
        m = {"xT": xTc.astype(BF16), "wv": wv_t, "wp": wp_t, "eb": eb}
        if USE_FP8:
            x8c = np.zeros((C, TOK_PAD8), np.float32)
            x8c[:, :TOK] = xc.T
            m["x8"] = np.ascontiguousarray(
                x8c.reshape(2, 2, 128, TOK_PAD8).transpose(2, 0, 1, 3)).astype(FP8)
            m["w8"] = w8
        else:
            m["wqk"] = wqk
        if has_bqk:
            m["bqk"] = bqk
        if has_bv:
            m["bv"] = bv
        if has_bp:
            m["bp"] = bp
        in_maps.append(m)
    return in_maps, has_bqk, has_bv, has_bp


def kernel(x, qkv_w, qkv_b, proj_w, proj_b, rpb_table, rel_index):
    from concourse import bacc
    from concourse.bass_utils import run_bass_kernel_spmd

    in_maps, has_bqk, has_bv, has_bp = _host_prep(
        np.asarray(x, np.float32), np.asarray(qkv_w, np.float32),
        np.asarray(qkv_b, np.float32), np.asarray(proj_w, np.float32),
        np.asarray(proj_b, np.float32), np.asarray(rpb_table, np.float32),
        np.asarray(rel_index))

    nc = bacc.Bacc()
    _build(nc, has_bqk, has_bv, has_bp)
    nc.finalize()

    trace = os.environ.get("BASS_KERNEL_TRACE", "") == "1"
    res = run_bass_kernel_spmd(nc, in_maps, core_ids=list(range(NCORES)),
                               trace=trace)
    if trace and res.exec_time_ns is not None:
        print(f"HW exec time: {res.exec_time_ns} ns", flush=True)

    outs = [r["out"].astype(np.float32).T.reshape(B_CORE, P, C)
            for r in res.results]
    return np.concatenate(outs, axis=0)
